# revision 1
# baseline (speedup 1.0000x reference)
"""Bass/Trainium2 kernel for nn_DecoderAttention (gnn message passing).

Math: q = query @ WQ.T is scattered to the 64 global nodes (glob_idx) and is
zero everywhere else, and the output only reads out[glob_idx].  Therefore only
edges whose dst is a global node contribute to the result.  Host-side we
partition the edge list by dst (CSR sort, as the sharding hint prescribes) and
shard the 64 global nodes across the 8 cores (node list i::8 -> core i).  Each
core gathers the <=CAP incoming edges of each of its 8 nodes with indirect
DMAs, projects the gathered x rows with K/V, does the per-node masked softmax
and aggregation, and applies the output projection for its 8 rows.

Fast path (64 glob ids, all < 127, cap 16 -- the spec's arange(64)): row_ptr
window [0:128) is loaded with one direct DMA and per-slot start/end offsets
come from constant selection matmuls, so the only data-dependent gathers are
src ids and x rows.  A general fallback using an indirect row_ptr gather
handles arbitrary glob_idx / larger caps.
"""

import os

import numpy as np

import concourse.bacc as bacc
import concourse.mybir as mybir
from concourse.bass import IndirectOffsetOnAxis
from concourse.bass_utils import run_bass_kernel_spmd
from concourse.masks import make_identity
from concourse.tile_rust import add_dep_helper
from concourse.tile import TileContext


class _SlimTailTileContext(TileContext):
    """TileContext whose kernel tail skips the final all-engine barrier.

    The standard tail is drain -> barrier -> sem clears -> barrier.  The last
    barrier only isolates the clears from code following the TileContext in
    multi-kernel modules; this NEFF ends right after, and each engine halts
    only once its own instruction stream (including the clears) completes, so
    it is dead weight here."""

    def _drain_and_barrier(self, tick_clock, wait_clock):
        from concourse.tile import ScopedClock

        nc = self.nc
        drain_inst = nc.sync.drain()
        wait_clock.add_sem_waits(
            drain_inst.ins, ScopedClock({None: tick_clock.global_clock})
        )
        # One drain->sem hop orders the gpsimd sem clears after all work,
        # instead of the full (expensive) all-engine EVSEM butterfly.
        done = nc.alloc_semaphore("tail_done")
        drain_inst.then_inc(done, 1)
        nc.gpsimd.wait_ge(done, 1)
        assert self.sems is not None
        popped = nc._tile_sem_poison_stack.pop()
        assert popped is self._sem_poison
        # sem_clear only (skip clear_and_free's dma_reset: each NEFF load
        # re-initializes the DMA rings, and the reset machinery is the
        # dominant cost of the kernel tail)
        from concourse.bass import compact_to_ranges
        nums = sorted(s.num if hasattr(s, "num") else s
                      for s in list(self.sems.allocated().values()) + [done])
        for r in compact_to_ranges(nums):
            nc.gpsimd.sem_clear(r)

D = 256
H = 4
DK = 64
NV = 40000
NE = 320000
B = 64
NCORES = 8
P = 128
NPC = B // NCORES  # nodes (output rows) per core: 8

F32 = mybir.dt.float32
I32 = mybir.dt.int32

_cache: dict = {}

last_results = None  # BassKernelResults of the most recent run (for harness)


def _build_fast(cap: int):
    """Fast-path SPMD program.

    Requires glob_idx == arange(64), cap == 16, and each core's shard
    (edges with dst % 8 == c, sorted by dst) to hold all edges of its 8
    global nodes within its first P entries.

    hdr per-core layout (128, WH):
        [:, 0:2*NPC]          qTm chunks (query rows c::8, transposed)
        [:, c_woff]           win_off (p % cap)
        [:, c_expj:+NPC]      expj  (slot->node lhsT)
        [0:NPC, c_expjt:+P]   expjt (node->slot lhsT)
        [0:NPC+1, c_rp]       shard-local row_ptr[j]   (j = 0..8)
        [0:NPC, c_rp+1]       shard-local row_ptr[j+1]
        [:, c_pref:+P]        2 * src id of shard edge p (replicated rows)
    """
    assert NPC * cap == P

    nc = bacc.Bacc("TRN2", target_bir_lowering=False, debug=False,
                   num_devices=NCORES)

    x_d = nc.dram_tensor("x", [2 * NV, D // 2], F32, kind="ExternalInput")
    wall_d = nc.dram_tensor("wall", [P, 8 * D], F32, kind="ExternalInput")
    WH = 2 * NPC + NPC + P + 3 + P
    hdr_d = nc.dram_tensor("hdr", [P, WH], F32, kind="ExternalInput")
    out_d = nc.dram_tensor("out_r", [NPC, D], F32, kind="ExternalOutput")

    c_expj = 2 * NPC
    c_expjt = c_expj + NPC
    c_offs = c_expjt + P
    c_pref = c_offs + 3

    with _SlimTailTileContext(nc) as tc:
        with (
            tc.tile_pool(name="sbuf", bufs=1) as sb,
            tc.tile_pool(name="psum", bufs=1, space="PSUM") as pp,
            tc.tile_pool(name="psmall", bufs=2, space="PSUM") as ps,
        ):
            hdr = sb.tile([P, WH], F32, tag="hdr")
            nc.sync.dma_start(out=hdr[:], in_=hdr_d[:])
            # wall on the scalar HWDGE ring: its sequencer is busy with the
            # ACT table load until ~8.5us, by which time hdr (sync ring) is
            # done -- so hdr's completion is not delayed by wall traffic.
            wall = sb.tile([P, 8 * D], F32, tag="wall")
            nc.scalar.dma_start(out=wall[:, 0:2 * D], in_=wall_d[:, 0:2 * D])
            nc.scalar.dma_start(out=wall[:, 2 * D:8 * D],
                                in_=wall_d[:, 2 * D:8 * D])

            ident = sb.tile([P, P], F32, tag="ident")
            make_identity(nc, ident[:])
            iota_t = sb.tile([P, P], F32, tag="iota_t")
            nc.gpsimd.iota(iota_t[:], pattern=[[1, P]], base=0,
                           channel_multiplier=0,
                           allow_small_or_imprecise_dtypes=True)

            ejt = hdr[0:NPC, c_expjt:c_expjt + P]
            ej = hdr[:, c_expj:c_expj + NPC]

            # per-slot offs/valid/exp-bias are host-precomputed hdr columns
            offs_f = hdr[:, c_offs:c_offs + 1]
            valid = hdr[:, c_offs + 1:c_offs + 2]
            negb = hdr[:, c_offs + 2:c_offs + 3]

            # src id selection from the in-hdr shard edge prefix:
            # srcA_f[p] = sum_w (iota[w] == offs[p]) * (2*src[w])
            sel_scr = sb.tile([P, P], F32, tag="sel_scr")
            srcA_f = sb.tile([P, 1], F32, tag="srcA_f")
            nc.vector.scalar_tensor_tensor(
                out=sel_scr[:], in0=iota_t[:], scalar=offs_f[:],
                in1=hdr[:, c_pref:c_pref + P], op0=mybir.AluOpType.is_equal,
                op1=mybir.AluOpType.mult, accum_out=srcA_f[:])
            srcB_f = sb.tile([P, 1], F32, tag="srcB_f")
            nc.vector.tensor_scalar_add(srcB_f[:], srcA_f[:], 1.0)
            srcA_i = sb.tile([P, 1], I32, tag="srcA_i")
            nc.vector.tensor_copy(out=srcA_i[:], in_=srcA_f[:])
            srcB_i = sb.tile([P, 1], I32, tag="srcB_i")
            nc.vector.tensor_copy(out=srcB_i[:], in_=srcB_f[:])

            # x row gathers, one per 128-wide half so downstream pipelines
            xselA = sb.tile([P, D // 2], F32, tag="xselA")
            nc.gpsimd.indirect_dma_start(
                out=xselA[:], out_offset=None, in_=x_d[:],
                in_offset=IndirectOffsetOnAxis(ap=srcA_i[:], axis=0))
            xselB = sb.tile([P, D // 2], F32, tag="xselB")
            nc.gpsimd.indirect_dma_start(
                out=xselB[:], out_offset=None, in_=x_d[:],
                in_offset=IndirectOffsetOnAxis(ap=srcB_i[:], axis=0))

            # q_mine = (query rows c::8) @ WQ.T   -> (NPC, D)
            qm_ps = ps.tile([NPC, D], F32, tag="ps_small")
            for t in range(2):
                nc.tensor.matmul(
                    out=qm_ps[:],
                    lhsT=hdr[:, t * NPC:(t + 1) * NPC],
                    rhs=wall[:, t * D:(t + 1) * D],
                    start=(t == 0), stop=(t == 1))
            qm = sb.tile([NPC, D], F32, tag="qm")
            nc.scalar.copy(out=qm[:], in_=qm_ps[:])

            # x_sel^T per half; K projection right after each half arrives
            xt_ps = pp.tile([P, D], F32, tag="ps_xt")
            xt = sb.tile([P, D], F32, tag="xt")
            k_ps = pp.tile([P, D], F32, tag="ps_k")
            v_ps = pp.tile([P, D], F32, tag="ps_v")
            k_insts = []
            for t, xh in enumerate((xselA, xselB)):
                nc.tensor.transpose(out=xt_ps[:, t * P:(t + 1) * P],
                                    in_=xh[:], identity=ident[:])
                nc.vector.tensor_copy(out=xt[:, t * P:(t + 1) * P],
                                      in_=xt_ps[:, t * P:(t + 1) * P])
                k_insts.append(nc.tensor.matmul(
                    out=k_ps[:],
                    lhsT=xt[:, t * P:(t + 1) * P],
                    rhs=wall[:, (1 + t) * 2 * D:(1 + t) * 2 * D + D],
                    start=(t == 0), stop=(t == 1)))
            v_insts = []
            for t in range(2):
                v_insts.append(nc.tensor.matmul(
                    out=v_ps[:],
                    lhsT=xt[:, t * P:(t + 1) * P],
                    rhs=wall[:, (1 + t) * 2 * D + D:(2 + t) * 2 * D],
                    start=(t == 0), stop=(t == 1)))
            add_dep_helper(v_insts[0].ins, k_insts[1].ins, sync=False,
                           reason="keep K completion ahead of V start")

            # qe = per-slot q row
            qe_ps = pp.tile([P, D], F32, tag="ps_qe")
            nc.tensor.matmul(out=qe_ps[:], lhsT=ejt, rhs=qm[:],
                             start=True, stop=True)
            qe = sb.tile([P, D], F32, tag="qe")
            nc.scalar.copy(out=qe[:], in_=qe_ps[:])

            # scores -> masked exp (mask folded into the exp bias)
            prod = sb.tile([P, D], F32, tag="prod")
            nc.vector.tensor_mul(out=prod[:], in0=qe[:], in1=k_ps[:])
            s = sb.tile([P, H], F32, tag="s")
            nc.vector.tensor_reduce(
                out=s[:], in_=prod[:].rearrange("p (h d) -> p h d", h=H),
                axis=mybir.AxisListType.X, op=mybir.AluOpType.add)
            agg = sb.tile([P, D + H + 1], F32, tag="agg")
            nc.scalar.activation(out=agg[:, D:D + H], in_=s[:],
                                 func=mybir.ActivationFunctionType.Exp,
                                 bias=negb[:],
                                 scale=float(1.0 / np.sqrt(DK)))
            nc.scalar.copy(out=agg[:, D + H:D + H + 1], in_=valid[:])
            nc.vector.tensor_tensor(
                out=agg[:, 0:D].rearrange("p (h d) -> p h d", h=H),
                in0=v_ps[:].rearrange("p (h d) -> p h d", h=H),
                in1=agg[:, D:D + H].to_broadcast([P, H, DK]),
                op=mybir.AluOpType.mult)

            # per-node reduction: [numer | denom | count]
            acc_ps = ps.tile([NPC, D + H + 1], F32, tag="ps_small")
            nc.tensor.matmul(out=acc_ps[:], lhsT=ej, rhs=agg[:],
                             start=True, stop=True)

            # normalize (guard empty nodes: denom += (count == 0))
            iszero = sb.tile([NPC, 1], F32, tag="iszero")
            nc.vector.tensor_scalar(out=iszero[:],
                                    in0=acc_ps[:, D + H:D + H + 1],
                                    scalar1=0.5, scalar2=None,
                                    op0=mybir.AluOpType.is_lt)
            den = sb.tile([NPC, H], F32, tag="den")
            nc.vector.tensor_scalar(out=den[:], in0=acc_ps[:, D:D + H],
                                    scalar1=iszero[:], scalar2=None,
                                    op0=mybir.AluOpType.add)
            rec = sb.tile([NPC, H], F32, tag="rec")
            nc.vector.reciprocal(out=rec[:], in_=den[:])
            onode = sb.tile([NPC, D], F32, tag="onode")
            nc.vector.tensor_tensor(
                out=onode[:].rearrange("p (h d) -> p h d", h=H),
                in0=acc_ps[:, 0:D].rearrange("p (h d) -> p h d", h=H),
                in1=rec[:].to_broadcast([NPC, H, DK]),
                op=mybir.AluOpType.mult)

            # r = out_node @ WO.T
            ot_ps = ps.tile([P, 2 * NPC], F32, tag="ps_small")
            for t in range(2):
                nc.tensor.transpose(out=ot_ps[:, t * NPC:(t + 1) * NPC],
                                    in_=onode[:, t * P:(t + 1) * P],
                                    identity=ident[0:NPC, 0:NPC])
            ot = sb.tile([P, 2 * NPC], F32, tag="ot")
            nc.vector.tensor_copy(out=ot[:], in_=ot_ps[:])
            r_ps = ps.tile([NPC, D], F32, tag="ps_small")
            for t in range(2):
                nc.tensor.matmul(out=r_ps[:],
                                 lhsT=ot[:, t * NPC:(t + 1) * NPC],
                                 rhs=wall[:, (6 + t) * D:(7 + t) * D],
                                 start=(t == 0), stop=(t == 1))
            r_sb = sb.tile([NPC, D], F32, tag="r_sb")
            nc.vector.tensor_copy(out=r_sb[:], in_=r_ps[:])
            nc.sync.dma_start(out=out_d[:], in_=r_sb[:])

    nc.compile()
    return nc


def _expanders(cap):
    nslots = NPC * cap
    nch = nslots // P
    npc_chunk = P // cap
    expjt = np.zeros((NPC, P * nch), np.float32)
    expj = np.zeros((P, NPC * nch), np.float32)
    for k in range(nch):
        j_of_p = np.arange(P) // cap + k * npc_chunk
        expjt[j_of_p, k * P + np.arange(P)] = 1.0
        expj[np.arange(P), k * NPC + j_of_p] = 1.0
    woff = (np.arange(P) % cap).astype(np.float32)
    return expjt, expj, woff, nch


def kernel(query, x, WQ, WK, WV, WO, src, dst, glob_idx):
    global last_results
    query = np.ascontiguousarray(np.asarray(query, dtype=np.float32))
    x = np.ascontiguousarray(np.asarray(x, dtype=np.float32))
    src32 = np.asarray(src, dtype=np.int32)
    dst32 = np.asarray(dst, dtype=np.int32)
    glob = np.asarray(glob_idx, dtype=np.int32)
    WQ = np.asarray(WQ, np.float32)
    WK = np.asarray(WK, np.float32)
    WV = np.asarray(WV, np.float32)
    WO = np.asarray(WO, np.float32)

    # partition (CSR-sort) edge list by dst shard (dst % 8), then dst
    shard = dst32 % NCORES
    order = np.lexsort((dst32, shard))
    s_src = src32[order]
    s_dst = dst32[order]
    s_shard = shard[order]
    shard_start = np.searchsorted(s_shard, np.arange(NCORES + 1))

    # per-global-node edge counts (for capacity + fast-path check)
    rel = dst32 < B
    gc = np.bincount(dst32[rel], minlength=B) if rel.any() else \
        np.zeros(B, np.int64)

    cap16_ok = gc.max() <= 16 if len(gc) else True
    pref_ok = all(gc[c::NCORES].sum() <= P for c in range(NCORES))
    fast = (np.array_equal(glob, np.arange(B, dtype=glob.dtype))
            and cap16_ok and pref_ok
            and not bool(int(os.environ.get("BASSK_FORCE_GENERAL", "0"))))

    if fast:
        res = _run_fast(query, x, s_src, s_dst, shard_start, WQ, WK, WV, WO)
    else:
        perm = np.argsort(dst32, kind="stable")
        sorted_src = np.ascontiguousarray(src32[perm])
        sorted_dst = dst32[perm]
        row_ptr = np.searchsorted(sorted_dst,
                                  np.arange(NV + 1)).astype(np.int32)
        gcnt = int((row_ptr[glob + 1] - row_ptr[glob]).max()) if len(glob) \
            else 0
        cap = 16
        while cap < gcnt:
            cap *= 2
        res = _run_general(query, x, sorted_src, row_ptr, glob, cap,
                           WQ, WK, WV, WO)
    last_results = res
    outs = [res.results[c]["out_r"] for c in range(NCORES)]
    return np.ascontiguousarray(
        np.stack(outs, axis=1).reshape(B, D).astype(np.float32))


def _run_fast(query, x, s_src, s_dst, shard_start, WQ, WK, WV, WO):
    cap = 16
    expjt, expj, woff, nch = _expanders(cap)
    assert nch == 1

    # weight wall: [wq0 wq1 | (wk|wv)0 (wk|wv)1 | wo0 wo1], chunks over d
    wqt, wkt, wvt, wot = WQ.T, WK.T, WV.T, WO.T
    wall = np.empty((P, 8 * D), np.float32)
    for t in range(2):
        dd = slice(t * P, (t + 1) * P)
        wall[:, t * D:(t + 1) * D] = wqt[dd]
        wall[:, (1 + t) * 2 * D:(1 + t) * 2 * D + D] = wkt[dd]
        wall[:, (1 + t) * 2 * D + D:(2 + t) * 2 * D] = wvt[dd]
        wall[:, (6 + t) * D:(7 + t) * D] = wot[dd]
    wall = np.ascontiguousarray(wall)

    shared = dict(x=np.ascontiguousarray(x.reshape(2 * NV, D // 2)),
                  wall=wall)

    WH = 2 * NPC + NPC + P + 3 + P
    c_expj = 2 * NPC
    c_expjt = c_expj + NPC
    c_offs = c_expjt + P
    c_pref = c_offs + 3

    qT = query.T  # (D, B)
    in_maps = []
    for c in range(NCORES):
        lo, hi = int(shard_start[c]), int(shard_start[c + 1])
        sh_dst = s_dst[lo:hi]
        sh_src = s_src[lo:hi]
        # shard-local row_ptr over my 8 nodes (c, c+8, .., c+56) + end
        my_nodes = c + NCORES * np.arange(NPC + 1)  # node c+64 bounds the end
        rp9 = np.searchsorted(sh_dst, my_nodes).astype(np.int64)
        nos = np.arange(P) // cap
        offs_col = rp9[nos] + np.arange(P) % cap
        valid_col = (offs_col < rp9[nos + 1]).astype(np.float32)
        hdr = np.zeros((P, WH), np.float32)
        for t in range(2):
            hdr[:, t * NPC:(t + 1) * NPC] = qT[t * P:(t + 1) * P, c::NCORES]
        hdr[:, c_expj:c_expj + NPC] = expj
        hdr[0:NPC, c_expjt:c_expjt + P] = expjt
        hdr[:, c_offs] = offs_col.astype(np.float32)
        hdr[:, c_offs + 1] = valid_col
        hdr[:, c_offs + 2] = (valid_col - 1.0) * 30.0
        n = min(P, hi - lo)
        prefrow = np.zeros(P, np.float32)
        prefrow[:n] = 2.0 * sh_src[:n]
        hdr[:, c_pref:c_pref + P] = prefrow[None, :]
        in_maps.append(dict(shared, hdr=np.ascontiguousarray(hdr)))

    key = ("fast", cap)
    if key not in _cache:
        _cache[key] = _build_fast(cap)
    nc = _cache[key]

    trace = bool(int(os.environ.get("BASSK_TRACE", "0")))
    return run_bass_kernel_spmd(nc, in_maps, core_ids=list(range(NCORES)),
                                trace=trace)


# ---------------------------------------------------------------------------
# general fallback (from validated v1 program)
# ---------------------------------------------------------------------------

def _build_general(cap: int):
    """Build the SPMD Bass program. cap = edge slots per node (power of two,
    NPC*cap multiple of 128)."""
    nslots = NPC * cap
    n_chunks = nslots // P
    assert nslots % P == 0
    npc_chunk = P // cap  # nodes per 128-slot chunk

    nc = bacc.Bacc("TRN2", target_bir_lowering=False, debug=False,
                   num_devices=NCORES)

    # ---- DRAM I/O ----
    x_d = nc.dram_tensor("x", [NV, D], F32, kind="ExternalInput")
    srcs_d = nc.dram_tensor("srcs", [NE + cap, 1], I32, kind="ExternalInput")
    rp_d = nc.dram_tensor("row_ptr", [NV + 1, 1], I32, kind="ExternalInput")
    qy_d = nc.dram_tensor("query", [B, D], F32, kind="ExternalInput")
    wqt_d = nc.dram_tensor("wqt", [D, D], F32, kind="ExternalInput")
    wkt_d = nc.dram_tensor("wkt", [D, D], F32, kind="ExternalInput")
    wvt_d = nc.dram_tensor("wvt", [D, D], F32, kind="ExternalInput")
    wot_d = nc.dram_tensor("wot", [D, D], F32, kind="ExternalInput")
    sel_d = nc.dram_tensor("sel", [B, NPC], F32, kind="ExternalInput")
    expjt_d = nc.dram_tensor("expjt", [NPC, P * n_chunks], F32,
                             kind="ExternalInput")
    expj_d = nc.dram_tensor("expj", [P, NPC * n_chunks], F32,
                            kind="ExternalInput")
    woff_d = nc.dram_tensor("win_off", [P, 1], F32, kind="ExternalInput")
    ident_d = nc.dram_tensor("ident", [P, P], F32, kind="ExternalInput")
    mgs_d = nc.dram_tensor("my_glob_s", [NPC, 1], I32, kind="ExternalInput")
    mge_d = nc.dram_tensor("my_glob_e", [NPC, 1], I32, kind="ExternalInput")
    out_d = nc.dram_tensor("out_r", [NPC, D], F32, kind="ExternalOutput")

    with _SlimTailTileContext(nc) as tc:
        with (
            tc.tile_pool(name="const", bufs=1) as cpool,
            tc.tile_pool(name="work", bufs=1) as wpool,
            tc.tile_pool(name="psum", bufs=1, space="PSUM") as ppool,
            tc.tile_pool(name="psum_small", bufs=2, space="PSUM") as spool,
        ):
            # ---- constant / weight loads (issued early, overlap the chain) --
            qy = cpool.tile([B, D], F32, tag="qy")
            nc.sync.dma_start(out=qy[:], in_=qy_d[:])
            wq = cpool.tile([P, 2 * D], F32, tag="wq")  # [d-chunk t] at cols t*D
            wk = cpool.tile([P, 2 * D], F32, tag="wk")
            wv = cpool.tile([P, 2 * D], F32, tag="wv")
            wo = cpool.tile([P, 2 * D], F32, tag="wo")
            for t in range(2):
                nc.sync.dma_start(out=wq[:, t * D:(t + 1) * D],
                                  in_=wqt_d[t * P:(t + 1) * P, :])
                nc.sync.dma_start(out=wk[:, t * D:(t + 1) * D],
                                  in_=wkt_d[t * P:(t + 1) * P, :])
                nc.sync.dma_start(out=wv[:, t * D:(t + 1) * D],
                                  in_=wvt_d[t * P:(t + 1) * P, :])
                nc.sync.dma_start(out=wo[:, t * D:(t + 1) * D],
                                  in_=wot_d[t * P:(t + 1) * P, :])
            sel = cpool.tile([B, NPC], F32, tag="sel")
            nc.sync.dma_start(out=sel[:], in_=sel_d[:])
            expjt = cpool.tile([NPC, P * n_chunks], F32, tag="expjt")
            nc.sync.dma_start(out=expjt[:], in_=expjt_d[:])
            expj = cpool.tile([P, NPC * n_chunks], F32, tag="expj")
            nc.sync.dma_start(out=expj[:], in_=expj_d[:])
            woff = cpool.tile([P, 1], F32, tag="woff")
            nc.sync.dma_start(out=woff[:], in_=woff_d[:])
            ident = cpool.tile([P, P], F32, tag="ident")
            nc.sync.dma_start(out=ident[:], in_=ident_d[:])
            mgs = cpool.tile([NPC, 1], I32, tag="mgs")
            nc.sync.dma_start(out=mgs[:], in_=mgs_d[:])
            mge = cpool.tile([NPC, 1], I32, tag="mge")
            nc.sync.dma_start(out=mge[:], in_=mge_d[:])

            # ---- row_ptr[glob] and row_ptr[glob+1] (one indirect gather) ----
            st_i = wpool.tile([NPC, 1], I32, tag="st_i")
            nc.gpsimd.indirect_dma_start(
                out=st_i[:], out_offset=None, in_=rp_d[:],
                in_offset=IndirectOffsetOnAxis(ap=mgs[:], axis=0))
            en_i = wpool.tile([NPC, 1], I32, tag="en_i")
            nc.gpsimd.indirect_dma_start(
                out=en_i[:], out_offset=None, in_=rp_d[:],
                in_offset=IndirectOffsetOnAxis(ap=mge[:], axis=0))
            st_f = wpool.tile([NPC, 1], F32, tag="st_f")
            nc.vector.tensor_copy(out=st_f[:], in_=st_i[:])
            en_f = wpool.tile([NPC, 1], F32, tag="en_f")
            nc.vector.tensor_copy(out=en_f[:], in_=en_i[:])

            # ---- q_glob = query @ WQ.T ; q_mine = my 8 rows ----
            qyt = wpool.tile([P, 2 * B], F32, tag="qyt")  # query^T d-chunks
            for t in range(2):
                pt = spool.tile([P, B], F32, tag="ps_small")
                nc.tensor.transpose(out=pt[:], in_=qy[:, t * P:(t + 1) * P],
                                    identity=ident[:B, :B])
                nc.vector.tensor_copy(out=qyt[:, t * B:(t + 1) * B], in_=pt[:])
            qg_ps = ppool.tile([B, D], F32, tag="ps_qg")
            for t in range(2):
                nc.tensor.matmul(out=qg_ps[:], lhsT=qyt[:, t * B:(t + 1) * B],
                                 rhs=wq[:, t * D:(t + 1) * D],
                                 start=(t == 0), stop=(t == 1))
            qg = wpool.tile([B, D], F32, tag="qg")
            nc.vector.tensor_copy(out=qg[:], in_=qg_ps[:])
            qm_ps = spool.tile([NPC, D], F32, tag="ps_small")
            nc.tensor.matmul(out=qm_ps[:], lhsT=sel[:], rhs=qg[:],
                             start=True, stop=True)
            qm = wpool.tile([NPC, D], F32, tag="qm")
            nc.vector.tensor_copy(out=qm[:], in_=qm_ps[:])

            # ---- accumulator over chunks (numer | denom | count) ----
            acc = wpool.tile([NPC, D + H + 1], F32, tag="acc")

            for k in range(n_chunks):
                ejt = expjt[:, k * P:(k + 1) * P]        # [NPC, P] lhsT
                ej = expj[:, k * NPC:(k + 1) * NPC]      # [P, NPC] lhsT

                # per-slot start/end expansion
                st_ps = spool.tile([P, 1], F32, tag="ps_small")
                en_ps = spool.tile([P, 1], F32, tag="ps_small")
                nc.tensor.matmul(out=st_ps[:], lhsT=ejt, rhs=st_f[:],
                                 start=True, stop=True)
                nc.tensor.matmul(out=en_ps[:], lhsT=ejt, rhs=en_f[:],
                                 start=True, stop=True)
                offs_f = wpool.tile([P, 1], F32, tag="offs_f")
                nc.vector.tensor_add(out=offs_f[:], in0=st_ps[:], in1=woff[:])
                valid = wpool.tile([P, 1], F32, tag="valid")
                nc.vector.tensor_tensor(out=valid[:], in0=offs_f[:],
                                        in1=en_ps[:], op=mybir.AluOpType.is_lt)
                offs_i = wpool.tile([P, 1], I32, tag="offs_i")
                nc.vector.tensor_copy(out=offs_i[:], in_=offs_f[:])

                # gather src ids, then x rows
                srcv = wpool.tile([P, 1], I32, tag="srcv")
                nc.gpsimd.indirect_dma_start(
                    out=srcv[:], out_offset=None, in_=srcs_d[:],
                    in_offset=IndirectOffsetOnAxis(ap=offs_i[:], axis=0))
                xsel = wpool.tile([P, D], F32, tag="xsel")
                nc.gpsimd.indirect_dma_start(
                    out=xsel[:], out_offset=None, in_=x_d[:],
                    in_offset=IndirectOffsetOnAxis(ap=srcv[:], axis=0))

                # x_sel^T (two 128x128 transposes)
                xt = wpool.tile([P, D], F32, tag="xt")
                for t in range(2):
                    xt_ps = spool.tile([P, P], F32, tag="ps_small")
                    nc.tensor.transpose(out=xt_ps[:],
                                        in_=xsel[:, t * P:(t + 1) * P],
                                        identity=ident[:])
                    nc.vector.tensor_copy(out=xt[:, t * P:(t + 1) * P],
                                          in_=xt_ps[:])

                # K/V projections of gathered rows
                k_ps = ppool.tile([P, D], F32, tag="ps_k")
                v_ps = ppool.tile([P, D], F32, tag="ps_v")
                for t in range(2):
                    nc.tensor.matmul(out=k_ps[:], lhsT=xt[:, t * P:(t + 1) * P],
                                     rhs=wk[:, t * D:(t + 1) * D],
                                     start=(t == 0), stop=(t == 1))
                for t in range(2):
                    nc.tensor.matmul(out=v_ps[:], lhsT=xt[:, t * P:(t + 1) * P],
                                     rhs=wv[:, t * D:(t + 1) * D],
                                     start=(t == 0), stop=(t == 1))
                ksel = wpool.tile([P, D], F32, tag="ksel")
                nc.vector.tensor_copy(out=ksel[:], in_=k_ps[:])
                vsel = wpool.tile([P, D], F32, tag="vsel")
                nc.vector.tensor_copy(out=vsel[:], in_=v_ps[:])

                # qe = q row per slot
                qe_ps = ppool.tile([P, D], F32, tag="ps_qe")
                nc.tensor.matmul(out=qe_ps[:], lhsT=ejt, rhs=qm[:],
                                 start=True, stop=True)

                # scores s[p,h], e = exp(s/8) * valid
                prod = wpool.tile([P, D], F32, tag="prod")
                nc.vector.tensor_mul(out=prod[:], in0=ksel[:], in1=qe_ps[:])
                s = wpool.tile([P, H], F32, tag="s")
                nc.vector.tensor_reduce(
                    out=s[:], in_=prod[:].rearrange("p (h d) -> p h d", h=H),
                    axis=mybir.AxisListType.X, op=mybir.AluOpType.add)
                e = wpool.tile([P, H], F32, tag="e")
                nc.scalar.activation(out=e[:], in_=s[:],
                                     func=mybir.ActivationFunctionType.Exp,
                                     scale=float(1.0 / np.sqrt(DK)))
                agg = wpool.tile([P, D + H + 1], F32, tag="agg")
                nc.vector.tensor_scalar_mul(agg[:, D:D + H], e[:], valid[:])
                nc.vector.tensor_copy(out=agg[:, D + H:D + H + 1], in_=valid[:])
                # w = v * alpha-weights (per head)
                for h in range(H):
                    nc.vector.tensor_scalar_mul(
                        agg[:, h * DK:(h + 1) * DK],
                        vsel[:, h * DK:(h + 1) * DK],
                        agg[:, D + h:D + h + 1])
                # per-node reduction (numer | denom | count)
                agg_ps = spool.tile([NPC, D + H + 1], F32, tag="ps_small")
                nc.tensor.matmul(out=agg_ps[:], lhsT=ej, rhs=agg[:],
                                 start=True, stop=True)
                if n_chunks == 1:
                    nc.vector.tensor_copy(out=acc[:], in_=agg_ps[:])
                elif k == 0:
                    nc.vector.tensor_copy(out=acc[:], in_=agg_ps[:])
                else:
                    nc.vector.tensor_add(out=acc[:], in0=acc[:], in1=agg_ps[:])

            # ---- normalize: out_node = numer / max(denom, empty-guard) ----
            iszero = wpool.tile([NPC, 1], F32, tag="iszero")
            nc.vector.tensor_scalar(out=iszero[:], in0=acc[:, D + H:D + H + 1],
                                    scalar1=0.5, scalar2=None,
                                    op0=mybir.AluOpType.is_lt)
            den = wpool.tile([NPC, H], F32, tag="den")
            nc.vector.tensor_scalar(out=den[:], in0=acc[:, D:D + H],
                                    scalar1=iszero[:], scalar2=None,
                                    op0=mybir.AluOpType.add)
            rec = wpool.tile([NPC, H], F32, tag="rec")
            nc.vector.reciprocal(out=rec[:], in_=den[:])
            onode = wpool.tile([NPC, D], F32, tag="onode")
            for h in range(H):
                nc.vector.tensor_scalar_mul(
                    onode[:, h * DK:(h + 1) * DK],
                    acc[:, h * DK:(h + 1) * DK], rec[:, h:h + 1])

            # ---- r = out_node @ WO.T ----
            ot = wpool.tile([P, 2 * NPC], F32, tag="ot")
            for t in range(2):
                ot_ps = spool.tile([P, NPC], F32, tag="ps_small")
                nc.tensor.transpose(out=ot_ps[:],
                                    in_=onode[:, t * P:(t + 1) * P],
                                    identity=ident[:NPC, :NPC])
                nc.vector.tensor_copy(out=ot[:, t * NPC:(t + 1) * NPC],
                                      in_=ot_ps[:])
            r_ps = spool.tile([NPC, D], F32, tag="ps_small")
            for t in range(2):
                nc.tensor.matmul(out=r_ps[:], lhsT=ot[:, t * NPC:(t + 1) * NPC],
                                 rhs=wo[:, t * D:(t + 1) * D],
                                 start=(t == 0), stop=(t == 1))
            r_sb = wpool.tile([NPC, D], F32, tag="r_sb")
            nc.vector.tensor_copy(out=r_sb[:], in_=r_ps[:])
            nc.sync.dma_start(out=out_d[:], in_=r_sb[:])

    nc.compile()
    return nc




def _run_general(query, x, sorted_src, row_ptr, glob, cap, WQ, WK, WV, WO):
    """General fallback: arbitrary glob_idx values / larger caps."""
    expjt, expj, woff, nch = _expanders(cap)
    srcs_pad = np.concatenate(
        [sorted_src, np.zeros(cap, np.int32)]).reshape(NE + cap, 1)
    rp2 = np.ascontiguousarray(row_ptr.reshape(NV + 1, 1))
    shared = dict(
        x=x, srcs=srcs_pad, row_ptr=rp2, query=query,
        wqt=np.ascontiguousarray(WQ.T), wkt=np.ascontiguousarray(WK.T),
        wvt=np.ascontiguousarray(WV.T), wot=np.ascontiguousarray(WO.T),
        expjt=expjt, expj=expj,
        win_off=np.ascontiguousarray(woff.reshape(P, 1)),
        ident=np.eye(P, dtype=np.float32))

    in_maps = []
    for c in range(NCORES):
        mine = glob[c::NCORES]
        mgs = mine.astype(np.int32).reshape(NPC, 1)
        mge = (mine + 1).astype(np.int32).reshape(NPC, 1)
        selc = np.zeros((B, NPC), np.float32)
        selc[c + NCORES * np.arange(NPC), np.arange(NPC)] = 1.0
        in_maps.append(dict(shared, my_glob_s=mgs, my_glob_e=mge, sel=selc))

    key = ("gen", cap)
    if key not in _cache:
        _cache[key] = _build_general(cap)
    nc = _cache[key]

    trace = bool(int(os.environ.get("BASSK_TRACE", "0")))
    return run_bass_kernel_spmd(nc, in_maps, core_ids=list(range(NCORES)),
                                trace=trace)



# revision 5
# speedup vs baseline: 1.4818x; 1.4818x over previous
"""Bass/Trainium2 kernel for nn_DecoderAttention (gnn message passing).

Math: q = query @ WQ.T is scattered to the 64 global nodes (glob_idx) and is
zero everywhere else, and the output only reads out[glob_idx].  Therefore only
edges whose dst is a global node contribute to the result.  Host-side we
partition the edge list by dst and shard the 64 global nodes across the 8
cores (node list c::8 -> core c).  Each core gathers the <=16 incoming edges
of each of its 8 nodes with one indirect DMA over a bf16 copy of x, projects
the gathered rows with K/V (bf16 matmuls, fp32 PSUM), does the per-node
masked softmax and aggregation, and applies the output projection for its 8
rows.

Fast path (glob_idx == arange(64), per-node edge count <= 16): the host
precomputes each slot's source row id directly (slot p = node p//16, edge
p%16), so the device's only data-dependent work is the single x-row gather.
Invalid slots get an exp bias of -100 (flushes their softmax weight to zero)
and the denominator gets +1e-30 so empty nodes produce exact zeros.  A
general fallback using an indirect row_ptr gather handles arbitrary
glob_idx / larger caps.
"""

import os

import ml_dtypes
import numpy as np

import concourse.bacc as bacc
import concourse.mybir as mybir
from concourse.bass import IndirectOffsetOnAxis
from concourse.bass_utils import run_bass_kernel_spmd
from concourse.tile import TileContext


class _SlimTailTileContext(TileContext):
    """TileContext whose kernel tail skips the final all-engine barrier.

    The standard tail is drain -> barrier -> sem clears -> barrier.  The last
    barrier only isolates the clears from code following the TileContext in
    multi-kernel modules; this NEFF ends right after, and each engine halts
    only once its own instruction stream (including the clears) completes, so
    it is dead weight here."""

    def _drain_and_barrier(self, tick_clock, wait_clock):
        from concourse.tile import ScopedClock

        nc = self.nc
        drain_inst = nc.sync.drain()
        wait_clock.add_sem_waits(
            drain_inst.ins, ScopedClock({None: tick_clock.global_clock})
        )
        # One drain->sem hop orders the gpsimd sem clears after all work,
        # instead of the full (expensive) all-engine EVSEM butterfly.
        done = nc.alloc_semaphore("tail_done")
        drain_inst.then_inc(done, 1)
        nc.gpsimd.wait_ge(done, 1)
        assert self.sems is not None
        popped = nc._tile_sem_poison_stack.pop()
        assert popped is self._sem_poison
        # sem_clear only (skip clear_and_free's dma_reset: each NEFF load
        # re-initializes the DMA rings, and the reset machinery is the
        # dominant cost of the kernel tail)
        from concourse.bass import compact_to_ranges
        nums = sorted(s.num if hasattr(s, "num") else s
                      for s in list(self.sems.allocated().values()) + [done])
        for r in compact_to_ranges(nums):
            nc.gpsimd.sem_clear(r)

D = 256
H = 4
DK = 64
NV = 40000
NE = 320000
B = 64
NCORES = 8
P = 128
NPC = B // NCORES  # nodes (output rows) per core: 8
CAP = 16           # edge slots per node in the fast path

F32 = mybir.dt.float32
I32 = mybir.dt.int32
BF16 = mybir.dt.bfloat16

# hdr column layout (f32 columns; bf16/i32 fields are bitcast views)
C_IDX = 0                 # [128, 1] i32 bits: x row id per slot
C_NEGB = 1                # [128, 1] f32: exp bias (0 valid, -100 invalid)
C_QT = 2                  # [128, 2*4] : qT bf16 chunks t=0,1, each [128,8]bf16
C_EXPJ = 10               # [128, 4]   : expj bf16 [128,8] (slot->node lhsT)
C_EXPJT = 14              # rows 0:8, [8, 64] : expjt bf16 [8,128] (node->slot)
HDR_W = 78

_cache: dict = {}

last_results = None  # BassKernelResults of the most recent run (for harness)


def _bf16_pack(a):
    """Pack a 2-D bf16-castable array into f32-bit columns (pairs of bf16)."""
    u = np.asarray(a, dtype=ml_dtypes.bfloat16).view(np.uint16)
    r, c = u.shape
    assert c % 2 == 0
    w = u.reshape(r, c // 2, 2)
    packed = w[:, :, 0].astype(np.uint32) | (w[:, :, 1].astype(np.uint32) << 16)
    return packed.view(np.float32)


def _build_fast():
    """Fast-path SPMD program (glob_idx == arange(64), cap 16)."""
    nc = bacc.Bacc("TRN2", target_bir_lowering=False, debug=False,
                   num_devices=NCORES)

    x_d = nc.dram_tensor("xbf", [NV, D], BF16, kind="ExternalInput")
    hdr_d = nc.dram_tensor("hdr", [P, HDR_W], F32, kind="ExternalInput")
    wq_d = nc.dram_tensor("wq", [P, 2 * D], BF16, kind="ExternalInput")
    wkv_d = nc.dram_tensor("wkv", [P, 4 * D + P], BF16, kind="ExternalInput")
    wo_d = nc.dram_tensor("wo", [P, 2 * D], BF16, kind="ExternalInput")
    out_d = nc.dram_tensor("out_r", [NPC, D], F32, kind="ExternalOutput")

    with _SlimTailTileContext(nc) as tc:
        with (
            tc.tile_pool(name="sbuf", bufs=1) as sb,
            tc.tile_pool(name="psum", bufs=1, space="PSUM") as pp,
            tc.tile_pool(name="psmall", bufs=2, space="PSUM") as ps,
        ):
            # ---- input DMAs.  sync ring: hdr (gather offsets) then wq.
            # scalar ring: wkv+ident (needed right after the gather) then wo.
            hdr = sb.tile([P, HDR_W], F32, tag="hdr")
            nc.sync.dma_start(out=hdr[:], in_=hdr_d[:])
            wq = sb.tile([P, 2 * D], BF16, tag="wq")
            nc.sync.dma_start(out=wq[:], in_=wq_d[:])
            wkv = sb.tile([P, 4 * D + P], BF16, tag="wkv")
            nc.scalar.dma_start(out=wkv[:], in_=wkv_d[:])
            wo = sb.tile([P, 2 * D], BF16, tag="wo")
            nc.scalar.dma_start(out=wo[:], in_=wo_d[:])
            ident = wkv[:, 4 * D:4 * D + P]

            # ---- the only data-dependent step: gather the slots' x rows
            xsel = sb.tile([P, D], BF16, tag="xsel")
            nc.gpsimd.indirect_dma_start(
                out=xsel[:], out_offset=None, in_=x_d[:],
                in_offset=IndirectOffsetOnAxis(ap=hdr[:, 0:1].bitcast(I32),
                                               axis=0))

            # ---- q_mine = (query rows c::8) @ WQ.T -> per-slot rows qe
            qm_ps = ps.tile([NPC, D], F32, tag="ps_small")
            for t in range(2):
                nc.tensor.matmul(
                    out=qm_ps[:],
                    lhsT=hdr[:, C_QT + 4 * t:C_QT + 4 * (t + 1)].bitcast(BF16),
                    rhs=wq[:, t * D:(t + 1) * D],
                    start=(t == 0), stop=(t == 1))
            qm = sb.tile([NPC, D], BF16, tag="qm")
            nc.scalar.copy(out=qm[:], in_=qm_ps[:])
            qe_ps = pp.tile([P, D], F32, tag="ps_qe")
            nc.tensor.matmul(
                out=qe_ps[:],
                lhsT=hdr[0:NPC, C_EXPJT:C_EXPJT + 64].bitcast(BF16),
                rhs=qm[:], start=True, stop=True)
            qe = sb.tile([P, D], F32, tag="qe")
            nc.scalar.copy(out=qe[:], in_=qe_ps[:])

            # ---- x_sel^T (bf16) then fused K|V projection into one bank
            xt_ps = pp.tile([P, D], BF16, tag="ps_xt")
            xt = sb.tile([P, D], BF16, tag="xt")
            for t in range(2):
                nc.tensor.transpose(out=xt_ps[:, t * P:(t + 1) * P],
                                    in_=xsel[:, t * P:(t + 1) * P],
                                    identity=ident)
                eng = nc.vector if t == 0 else nc.scalar
                if t == 0:
                    eng.tensor_copy(out=xt[:, t * P:(t + 1) * P],
                                    in_=xt_ps[:, t * P:(t + 1) * P])
                else:
                    eng.copy(out=xt[:, t * P:(t + 1) * P],
                             in_=xt_ps[:, t * P:(t + 1) * P])
            kv_ps = pp.tile([P, 2 * D], F32, tag="ps_kv")
            for t in range(2):
                nc.tensor.matmul(out=kv_ps[:],
                                 lhsT=xt[:, t * P:(t + 1) * P],
                                 rhs=wkv[:, t * 2 * D:(t + 1) * 2 * D],
                                 start=(t == 0), stop=(t == 1))
            k_ps = kv_ps[:, 0:D]
            v_ps = kv_ps[:, D:2 * D]

            # ---- scores -> masked exp (mask folded into the exp bias)
            prod = sb.tile([P, D], F32, tag="prod")
            nc.vector.tensor_mul(out=prod[:], in0=qe[:], in1=k_ps)
            s = sb.tile([P, H], F32, tag="s")
            nc.vector.tensor_reduce(
                out=s[:], in_=prod[:].rearrange("p (h d) -> p h d", h=H),
                axis=mybir.AxisListType.X, op=mybir.AluOpType.add)
            agg = sb.tile([P, D + H], BF16, tag="agg")
            nc.scalar.activation(out=agg[:, D:D + H], in_=s[:],
                                 func=mybir.ActivationFunctionType.Exp,
                                 bias=hdr[:, C_NEGB:C_NEGB + 1],
                                 scale=float(1.0 / np.sqrt(DK)))
            nc.vector.tensor_tensor(
                out=agg[:, 0:D].rearrange("p (h d) -> p h d", h=H),
                in0=v_ps.rearrange("p (h d) -> p h d", h=H),
                in1=agg[:, D:D + H].to_broadcast([P, H, DK]),
                op=mybir.AluOpType.mult)

            # ---- per-node reduction: [numer | denom]
            acc_ps = ps.tile([NPC, D + H], F32, tag="ps_small")
            nc.tensor.matmul(out=acc_ps[:],
                             lhsT=hdr[:, C_EXPJ:C_EXPJ + 4].bitcast(BF16),
                             rhs=agg[:], start=True, stop=True)

            # ---- normalize (+1e-30 so empty nodes give exact zeros)
            den = sb.tile([NPC, H], F32, tag="den")
            nc.vector.tensor_scalar_add(den[:], acc_ps[:, D:D + H], 1e-30)
            rec = sb.tile([NPC, H], F32, tag="rec")
            nc.vector.reciprocal(out=rec[:], in_=den[:])
            onode = sb.tile([NPC, D], BF16, tag="onode")
            nc.vector.tensor_tensor(
                out=onode[:].rearrange("p (h d) -> p h d", h=H),
                in0=acc_ps[:, 0:D].rearrange("p (h d) -> p h d", h=H),
                in1=rec[:].to_broadcast([NPC, H, DK]),
                op=mybir.AluOpType.mult)

            # ---- r = out_node @ WO.T
            ot_ps = ps.tile([P, 2 * NPC], BF16, tag="ps_ot")
            for t in range(2):
                nc.tensor.transpose(out=ot_ps[:, t * NPC:(t + 1) * NPC],
                                    in_=onode[:, t * P:(t + 1) * P],
                                    identity=ident[0:NPC, 0:NPC])
            ot = sb.tile([P, 2 * NPC], BF16, tag="ot")
            nc.vector.tensor_copy(out=ot[:], in_=ot_ps[:])
            r_ps = ps.tile([NPC, D], F32, tag="ps_small")
            for t in range(2):
                nc.tensor.matmul(out=r_ps[:],
                                 lhsT=ot[:, t * NPC:(t + 1) * NPC],
                                 rhs=wo[:, t * D:(t + 1) * D],
                                 start=(t == 0), stop=(t == 1))
            r_sb = sb.tile([NPC, D], F32, tag="r_sb")
            nc.vector.tensor_copy(out=r_sb[:], in_=r_ps[:])
            nc.sync.dma_start(out=out_d[:], in_=r_sb[:])

    nc.compile()
    return nc


def kernel(query, x, WQ, WK, WV, WO, src, dst, glob_idx):
    global last_results
    query = np.ascontiguousarray(np.asarray(query, dtype=np.float32))
    x = np.ascontiguousarray(np.asarray(x, dtype=np.float32))
    src32 = np.asarray(src, dtype=np.int32)
    dst32 = np.asarray(dst, dtype=np.int32)
    glob = np.asarray(glob_idx, dtype=np.int32)
    WQ = np.asarray(WQ, np.float32)
    WK = np.asarray(WK, np.float32)
    WV = np.asarray(WV, np.float32)
    WO = np.asarray(WO, np.float32)

    # per-global-node edge counts (for capacity + fast-path check)
    rel = dst32 < B
    gc = np.bincount(dst32[rel], minlength=B) if rel.any() else \
        np.zeros(B, np.int64)

    fast = (np.array_equal(glob, np.arange(B, dtype=glob.dtype))
            and (gc.max() <= CAP if len(gc) else True)
            and not bool(int(os.environ.get("BASSK_FORCE_GENERAL", "0"))))

    if fast:
        res = _run_fast(query, x, src32, dst32, WQ, WK, WV, WO)
    else:
        perm = np.argsort(dst32, kind="stable")
        sorted_src = np.ascontiguousarray(src32[perm])
        sorted_dst = dst32[perm]
        row_ptr = np.searchsorted(sorted_dst,
                                  np.arange(NV + 1)).astype(np.int32)
        gcnt = int((row_ptr[glob + 1] - row_ptr[glob]).max()) if len(glob) \
            else 0
        cap = 16
        while cap < gcnt:
            cap *= 2
        res = _run_general(query, x, sorted_src, row_ptr, glob, cap,
                           WQ, WK, WV, WO)
    last_results = res
    outs = [res.results[c]["out_r"] for c in range(NCORES)]
    return np.ascontiguousarray(
        np.stack(outs, axis=1).reshape(B, D).astype(np.float32))


def _run_fast(query, x, src32, dst32, WQ, WK, WV, WO):
    # only edges into the 64 global nodes matter; sort those by dst
    rel = np.flatnonzero(dst32 < B)
    r_dst = dst32[rel]
    order = np.argsort(r_dst, kind="stable")
    s_dst = r_dst[order]
    s_src = src32[rel][order]

    ident = np.eye(P, dtype=np.float32)
    wall_wq = np.empty((P, 2 * D), np.float32)
    wall_kv = np.empty((P, 4 * D + P), np.float32)
    wall_wo = np.empty((P, 2 * D), np.float32)
    wqt, wkt, wvt, wot = WQ.T, WK.T, WV.T, WO.T
    for t in range(2):
        dd = slice(t * P, (t + 1) * P)
        wall_wq[:, t * D:(t + 1) * D] = wqt[dd]
        wall_kv[:, t * 2 * D:t * 2 * D + D] = wkt[dd]
        wall_kv[:, t * 2 * D + D:(t + 1) * 2 * D] = wvt[dd]
        wall_wo[:, t * D:(t + 1) * D] = wot[dd]
    wall_kv[:, 4 * D:] = ident

    bf = ml_dtypes.bfloat16
    shared = dict(
        xbf=np.ascontiguousarray(x.astype(bf)),
        wq=np.ascontiguousarray(wall_wq.astype(bf)),
        wkv=np.ascontiguousarray(wall_kv.astype(bf)),
        wo=np.ascontiguousarray(wall_wo.astype(bf)),
    )

    # expanders: slot p belongs to node j = p // CAP
    j_of_p = np.arange(P) // CAP
    expjt = np.zeros((NPC, P), np.float32)
    expjt[j_of_p, np.arange(P)] = 1.0
    expj = np.zeros((P, NPC), np.float32)
    expj[np.arange(P), j_of_p] = 1.0

    qT = query.T  # (D, B)
    in_maps = []
    for c in range(NCORES):
        my_nodes = c + NCORES * np.arange(NPC)
        lo = np.searchsorted(s_dst, my_nodes)
        hi = np.searchsorted(s_dst, my_nodes + 1)
        offs = lo[j_of_p] + np.arange(P) % CAP
        valid = offs < hi[j_of_p]
        idx = np.where(valid, s_src[np.minimum(offs, len(s_src) - 1)]
                       if len(s_src) else 0, 0).astype(np.int32)

        hdr = np.zeros((P, HDR_W), np.float32)
        hdr[:, C_IDX] = idx.view(np.float32)
        hdr[:, C_NEGB] = np.where(valid, 0.0, -100.0).astype(np.float32)
        for t in range(2):
            hdr[:, C_QT + 4 * t:C_QT + 4 * (t + 1)] = _bf16_pack(
                qT[t * P:(t + 1) * P, c::NCORES])
        hdr[:, C_EXPJ:C_EXPJ + 4] = _bf16_pack(expj)
        hdr[0:NPC, C_EXPJT:C_EXPJT + 64] = _bf16_pack(expjt)
        in_maps.append(dict(shared, hdr=np.ascontiguousarray(hdr)))

    key = "fastv2"
    if key not in _cache:
        _cache[key] = _build_fast()
    nc = _cache[key]

    trace = bool(int(os.environ.get("BASSK_TRACE", "0")))
    return run_bass_kernel_spmd(nc, in_maps, core_ids=list(range(NCORES)),
                                trace=trace)


# ---------------------------------------------------------------------------
# general fallback (from validated v1 program)
# ---------------------------------------------------------------------------

def _expanders(cap):
    nslots = NPC * cap
    nch = nslots // P
    npc_chunk = P // cap
    expjt = np.zeros((NPC, P * nch), np.float32)
    expj = np.zeros((P, NPC * nch), np.float32)
    for k in range(nch):
        j_of_p = np.arange(P) // cap + k * npc_chunk
        expjt[j_of_p, k * P + np.arange(P)] = 1.0
        expj[np.arange(P), k * NPC + j_of_p] = 1.0
    woff = (np.arange(P) % cap).astype(np.float32)
    return expjt, expj, woff, nch


def _build_general(cap: int):
    """Build the SPMD Bass program. cap = edge slots per node (power of two,
    NPC*cap multiple of 128)."""
    nslots = NPC * cap
    n_chunks = nslots // P
    assert nslots % P == 0

    nc = bacc.Bacc("TRN2", target_bir_lowering=False, debug=False,
                   num_devices=NCORES)

    # ---- DRAM I/O ----
    x_d = nc.dram_tensor("x", [NV, D], F32, kind="ExternalInput")
    srcs_d = nc.dram_tensor("srcs", [NE + cap, 1], I32, kind="ExternalInput")
    rp_d = nc.dram_tensor("row_ptr", [NV + 1, 1], I32, kind="ExternalInput")
    qy_d = nc.dram_tensor("query", [B, D], F32, kind="ExternalInput")
    wqt_d = nc.dram_tensor("wqt", [D, D], F32, kind="ExternalInput")
    wkt_d = nc.dram_tensor("wkt", [D, D], F32, kind="ExternalInput")
    wvt_d = nc.dram_tensor("wvt", [D, D], F32, kind="ExternalInput")
    wot_d = nc.dram_tensor("wot", [D, D], F32, kind="ExternalInput")
    sel_d = nc.dram_tensor("sel", [B, NPC], F32, kind="ExternalInput")
    expjt_d = nc.dram_tensor("expjt", [NPC, P * n_chunks], F32,
                             kind="ExternalInput")
    expj_d = nc.dram_tensor("expj", [P, NPC * n_chunks], F32,
                            kind="ExternalInput")
    woff_d = nc.dram_tensor("win_off", [P, 1], F32, kind="ExternalInput")
    ident_d = nc.dram_tensor("ident", [P, P], F32, kind="ExternalInput")
    mgs_d = nc.dram_tensor("my_glob_s", [NPC, 1], I32, kind="ExternalInput")
    mge_d = nc.dram_tensor("my_glob_e", [NPC, 1], I32, kind="ExternalInput")
    out_d = nc.dram_tensor("out_r", [NPC, D], F32, kind="ExternalOutput")

    with _SlimTailTileContext(nc) as tc:
        with (
            tc.tile_pool(name="const", bufs=1) as cpool,
            tc.tile_pool(name="work", bufs=1) as wpool,
            tc.tile_pool(name="psum", bufs=1, space="PSUM") as ppool,
            tc.tile_pool(name="psum_small", bufs=2, space="PSUM") as spool,
        ):
            # ---- constant / weight loads (issued early, overlap the chain) --
            qy = cpool.tile([B, D], F32, tag="qy")
            nc.sync.dma_start(out=qy[:], in_=qy_d[:])
            wq = cpool.tile([P, 2 * D], F32, tag="wq")  # [d-chunk t] at cols t*D
            wk = cpool.tile([P, 2 * D], F32, tag="wk")
            wv = cpool.tile([P, 2 * D], F32, tag="wv")
            wo = cpool.tile([P, 2 * D], F32, tag="wo")
            for t in range(2):
                nc.sync.dma_start(out=wq[:, t * D:(t + 1) * D],
                                  in_=wqt_d[t * P:(t + 1) * P, :])
                nc.sync.dma_start(out=wk[:, t * D:(t + 1) * D],
                                  in_=wkt_d[t * P:(t + 1) * P, :])
                nc.sync.dma_start(out=wv[:, t * D:(t + 1) * D],
                                  in_=wvt_d[t * P:(t + 1) * P, :])
                nc.sync.dma_start(out=wo[:, t * D:(t + 1) * D],
                                  in_=wot_d[t * P:(t + 1) * P, :])
            sel = cpool.tile([B, NPC], F32, tag="sel")
            nc.sync.dma_start(out=sel[:], in_=sel_d[:])
            expjt = cpool.tile([NPC, P * n_chunks], F32, tag="expjt")
            nc.sync.dma_start(out=expjt[:], in_=expjt_d[:])
            expj = cpool.tile([P, NPC * n_chunks], F32, tag="expj")
            nc.sync.dma_start(out=expj[:], in_=expj_d[:])
            woff = cpool.tile([P, 1], F32, tag="woff")
            nc.sync.dma_start(out=woff[:], in_=woff_d[:])
            ident = cpool.tile([P, P], F32, tag="ident")
            nc.sync.dma_start(out=ident[:], in_=ident_d[:])
            mgs = cpool.tile([NPC, 1], I32, tag="mgs")
            nc.sync.dma_start(out=mgs[:], in_=mgs_d[:])
            mge = cpool.tile([NPC, 1], I32, tag="mge")
            nc.sync.dma_start(out=mge[:], in_=mge_d[:])

            # ---- row_ptr[glob] and row_ptr[glob+1] (one indirect gather) ----
            st_i = wpool.tile([NPC, 1], I32, tag="st_i")
            nc.gpsimd.indirect_dma_start(
                out=st_i[:], out_offset=None, in_=rp_d[:],
                in_offset=IndirectOffsetOnAxis(ap=mgs[:], axis=0))
            en_i = wpool.tile([NPC, 1], I32, tag="en_i")
            nc.gpsimd.indirect_dma_start(
                out=en_i[:], out_offset=None, in_=rp_d[:],
                in_offset=IndirectOffsetOnAxis(ap=mge[:], axis=0))
            st_f = wpool.tile([NPC, 1], F32, tag="st_f")
            nc.vector.tensor_copy(out=st_f[:], in_=st_i[:])
            en_f = wpool.tile([NPC, 1], F32, tag="en_f")
            nc.vector.tensor_copy(out=en_f[:], in_=en_i[:])

            # ---- q_glob = query @ WQ.T ; q_mine = my 8 rows ----
            qyt = wpool.tile([P, 2 * B], F32, tag="qyt")  # query^T d-chunks
            for t in range(2):
                pt = spool.tile([P, B], F32, tag="ps_small")
                nc.tensor.transpose(out=pt[:], in_=qy[:, t * P:(t + 1) * P],
                                    identity=ident[:B, :B])
                nc.vector.tensor_copy(out=qyt[:, t * B:(t + 1) * B], in_=pt[:])
            qg_ps = ppool.tile([B, D], F32, tag="ps_qg")
            for t in range(2):
                nc.tensor.matmul(out=qg_ps[:], lhsT=qyt[:, t * B:(t + 1) * B],
                                 rhs=wq[:, t * D:(t + 1) * D],
                                 start=(t == 0), stop=(t == 1))
            qg = wpool.tile([B, D], F32, tag="qg")
            nc.vector.tensor_copy(out=qg[:], in_=qg_ps[:])
            qm_ps = spool.tile([NPC, D], F32, tag="ps_small")
            nc.tensor.matmul(out=qm_ps[:], lhsT=sel[:], rhs=qg[:],
                             start=True, stop=True)
            qm = wpool.tile([NPC, D], F32, tag="qm")
            nc.vector.tensor_copy(out=qm[:], in_=qm_ps[:])

            # ---- accumulator over chunks (numer | denom | count) ----
            acc = wpool.tile([NPC, D + H + 1], F32, tag="acc")

            for k in range(n_chunks):
                ejt = expjt[:, k * P:(k + 1) * P]        # [NPC, P] lhsT
                ej = expj[:, k * NPC:(k + 1) * NPC]      # [P, NPC] lhsT

                # per-slot start/end expansion
                st_ps = spool.tile([P, 1], F32, tag="ps_small")
                en_ps = spool.tile([P, 1], F32, tag="ps_small")
                nc.tensor.matmul(out=st_ps[:], lhsT=ejt, rhs=st_f[:],
                                 start=True, stop=True)
                nc.tensor.matmul(out=en_ps[:], lhsT=ejt, rhs=en_f[:],
                                 start=True, stop=True)
                offs_f = wpool.tile([P, 1], F32, tag="offs_f")
                nc.vector.tensor_add(out=offs_f[:], in0=st_ps[:], in1=woff[:])
                valid = wpool.tile([P, 1], F32, tag="valid")
                nc.vector.tensor_tensor(out=valid[:], in0=offs_f[:],
                                        in1=en_ps[:], op=mybir.AluOpType.is_lt)
                offs_i = wpool.tile([P, 1], I32, tag="offs_i")
                nc.vector.tensor_copy(out=offs_i[:], in_=offs_f[:])

                # gather src ids, then x rows
                srcv = wpool.tile([P, 1], I32, tag="srcv")
                nc.gpsimd.indirect_dma_start(
                    out=srcv[:], out_offset=None, in_=srcs_d[:],
                    in_offset=IndirectOffsetOnAxis(ap=offs_i[:], axis=0))
                xsel = wpool.tile([P, D], F32, tag="xsel")
                nc.gpsimd.indirect_dma_start(
                    out=xsel[:], out_offset=None, in_=x_d[:],
                    in_offset=IndirectOffsetOnAxis(ap=srcv[:], axis=0))

                # x_sel^T (two 128x128 transposes)
                xt = wpool.tile([P, D], F32, tag="xt")
                for t in range(2):
                    xt_ps = spool.tile([P, P], F32, tag="ps_small")
                    nc.tensor.transpose(out=xt_ps[:],
                                        in_=xsel[:, t * P:(t + 1) * P],
                                        identity=ident[:])
                    nc.vector.tensor_copy(out=xt[:, t * P:(t + 1) * P],
                                          in_=xt_ps[:])

                # K/V projections of gathered rows
                k_ps = ppool.tile([P, D], F32, tag="ps_k")
                v_ps = ppool.tile([P, D], F32, tag="ps_v")
                for t in range(2):
                    nc.tensor.matmul(out=k_ps[:], lhsT=xt[:, t * P:(t + 1) * P],
                                     rhs=wk[:, t * D:(t + 1) * D],
                                     start=(t == 0), stop=(t == 1))
                for t in range(2):
                    nc.tensor.matmul(out=v_ps[:], lhsT=xt[:, t * P:(t + 1) * P],
                                     rhs=wv[:, t * D:(t + 1) * D],
                                     start=(t == 0), stop=(t == 1))
                ksel = wpool.tile([P, D], F32, tag="ksel")
                nc.vector.tensor_copy(out=ksel[:], in_=k_ps[:])
                vsel = wpool.tile([P, D], F32, tag="vsel")
                nc.vector.tensor_copy(out=vsel[:], in_=v_ps[:])

                # qe = q row per slot
                qe_ps = ppool.tile([P, D], F32, tag="ps_qe")
                nc.tensor.matmul(out=qe_ps[:], lhsT=ejt, rhs=qm[:],
                                 start=True, stop=True)

                # scores s[p,h], e = exp(s/8) * valid
                prod = wpool.tile([P, D], F32, tag="prod")
                nc.vector.tensor_mul(out=prod[:], in0=ksel[:], in1=qe_ps[:])
                s = wpool.tile([P, H], F32, tag="s")
                nc.vector.tensor_reduce(
                    out=s[:], in_=prod[:].rearrange("p (h d) -> p h d", h=H),
                    axis=mybir.AxisListType.X, op=mybir.AluOpType.add)
                e = wpool.tile([P, H], F32, tag="e")
                nc.scalar.activation(out=e[:], in_=s[:],
                                     func=mybir.ActivationFunctionType.Exp,
                                     scale=float(1.0 / np.sqrt(DK)))
                agg = wpool.tile([P, D + H + 1], F32, tag="agg")
                nc.vector.tensor_scalar_mul(agg[:, D:D + H], e[:], valid[:])
                nc.vector.tensor_copy(out=agg[:, D + H:D + H + 1], in_=valid[:])
                # w = v * alpha-weights (per head)
                for h in range(H):
                    nc.vector.tensor_scalar_mul(
                        agg[:, h * DK:(h + 1) * DK],
                        vsel[:, h * DK:(h + 1) * DK],
                        agg[:, D + h:D + h + 1])
                # per-node reduction (numer | denom | count)
                agg_ps = spool.tile([NPC, D + H + 1], F32, tag="ps_small")
                nc.tensor.matmul(out=agg_ps[:], lhsT=ej, rhs=agg[:],
                                 start=True, stop=True)
                if n_chunks == 1:
                    nc.vector.tensor_copy(out=acc[:], in_=agg_ps[:])
                elif k == 0:
                    nc.vector.tensor_copy(out=acc[:], in_=agg_ps[:])
                else:
                    nc.vector.tensor_add(out=acc[:], in0=acc[:], in1=agg_ps[:])

            # ---- normalize: out_node = numer / max(denom, empty-guard) ----
            iszero = wpool.tile([NPC, 1], F32, tag="iszero")
            nc.vector.tensor_scalar(out=iszero[:], in0=acc[:, D + H:D + H + 1],
                                    scalar1=0.5, scalar2=None,
                                    op0=mybir.AluOpType.is_lt)
            den = wpool.tile([NPC, H], F32, tag="den")
            nc.vector.tensor_scalar(out=den[:], in0=acc[:, D:D + H],
                                    scalar1=iszero[:], scalar2=None,
                                    op0=mybir.AluOpType.add)
            rec = wpool.tile([NPC, H], F32, tag="rec")
            nc.vector.reciprocal(out=rec[:], in_=den[:])
            onode = wpool.tile([NPC, D], F32, tag="onode")
            for h in range(H):
                nc.vector.tensor_scalar_mul(
                    onode[:, h * DK:(h + 1) * DK],
                    acc[:, h * DK:(h + 1) * DK], rec[:, h:h + 1])

            # ---- r = out_node @ WO.T ----
            ot = wpool.tile([P, 2 * NPC], F32, tag="ot")
            for t in range(2):
                ot_ps = spool.tile([P, NPC], F32, tag="ps_small")
                nc.tensor.transpose(out=ot_ps[:],
                                    in_=onode[:, t * P:(t + 1) * P],
                                    identity=ident[:NPC, :NPC])
                nc.vector.tensor_copy(out=ot[:, t * NPC:(t + 1) * NPC],
                                      in_=ot_ps[:])
            r_ps = spool.tile([NPC, D], F32, tag="ps_small")
            for t in range(2):
                nc.tensor.matmul(out=r_ps[:], lhsT=ot[:, t * NPC:(t + 1) * NPC],
                                 rhs=wo[:, t * D:(t + 1) * D],
                                 start=(t == 0), stop=(t == 1))
            r_sb = wpool.tile([NPC, D], F32, tag="r_sb")
            nc.vector.tensor_copy(out=r_sb[:], in_=r_ps[:])
            nc.sync.dma_start(out=out_d[:], in_=r_sb[:])

    nc.compile()
    return nc


def _run_general(query, x, sorted_src, row_ptr, glob, cap, WQ, WK, WV, WO):
    """General fallback: arbitrary glob_idx values / larger caps."""
    expjt, expj, woff, nch = _expanders(cap)
    srcs_pad = np.concatenate(
        [sorted_src, np.zeros(cap, np.int32)]).reshape(NE + cap, 1)
    rp2 = np.ascontiguousarray(row_ptr.reshape(NV + 1, 1))
    shared = dict(
        x=x, srcs=srcs_pad, row_ptr=rp2, query=query,
        wqt=np.ascontiguousarray(WQ.T), wkt=np.ascontiguousarray(WK.T),
        wvt=np.ascontiguousarray(WV.T), wot=np.ascontiguousarray(WO.T),
        expjt=expjt, expj=expj,
        win_off=np.ascontiguousarray(woff.reshape(P, 1)),
        ident=np.eye(P, dtype=np.float32))

    in_maps = []
    for c in range(NCORES):
        mine = glob[c::NCORES]
        mgs = mine.astype(np.int32).reshape(NPC, 1)
        mge = (mine + 1).astype(np.int32).reshape(NPC, 1)
        selc = np.zeros((B, NPC), np.float32)
        selc[c + NCORES * np.arange(NPC), np.arange(NPC)] = 1.0
        in_maps.append(dict(shared, my_glob_s=mgs, my_glob_e=mge, sel=selc))

    key = ("gen", cap)
    if key not in _cache:
        _cache[key] = _build_general(cap)
    nc = _cache[key]

    trace = bool(int(os.environ.get("BASSK_TRACE", "0")))
    return run_bass_kernel_spmd(nc, in_maps, core_ids=list(range(NCORES)),
                                trace=trace)


# revision 12
# speedup vs baseline: 1.4992x; 1.0118x over previous
"""Bass/Trainium2 kernel for nn_DecoderAttention (gnn message passing).

Math: q = query @ WQ.T is scattered to the 64 global nodes (glob_idx) and is
zero everywhere else, and the output only reads out[glob_idx].  Therefore only
edges whose dst is a global node contribute to the result.  Host-side we
partition the edge list by dst and shard the 64 global nodes across the 8
cores (node list c::8 -> core c).  Each core gathers the <=16 incoming edges
of each of its 8 nodes with one indirect DMA over a bf16 copy of x, computes
the per-edge scores against WK-folded queries, does the per-node masked
softmax and V aggregation (bf16 matmuls, fp32 PSUM), and applies the output
projection for its 8 rows.

Fast path (glob_idx == arange(64), per-node edge count <= 16): the host
precomputes each slot's source row id directly (slot p = node p//16, edge
p%16), so the device's only data-dependent work is the single x-row gather.
Scores use the fold  s[p,h] = x[src_p] . T[(node_p,h)]  with
T[(n,h),:] = sum_{o in head h} q[n,o] WK[o,:], so no K projection of the
gathered rows is needed.  Invalid slots get an exp bias of -100 (flushes
their softmax weight to zero) and the denominator gets +1e-30 so empty nodes
produce exact zeros.  Dummy matmuls keep the PE busy while DMAs are in
flight so the real matmuls run at full (ramped) clock.  A general fallback
using an indirect row_ptr gather handles arbitrary glob_idx / larger caps.
"""

import os

import ml_dtypes
import numpy as np

import concourse.bacc as bacc
import concourse.mybir as mybir
from concourse.bass import IndirectOffsetOnAxis
from concourse.bass_utils import run_bass_kernel_spmd
from concourse.tile import TileContext


class _SlimTailTileContext(TileContext):
    """TileContext whose kernel tail is just a drain.

    The standard tail is drain -> barrier -> sem clears -> barrier.  The NRT
    execution epilogue zeroes the entire semaphore file after every execute,
    so the kernel's own clears are redundant; only the drain (which holds the
    NEFF open until the output DMA lands) is load-bearing."""

    def _drain_and_barrier(self, tick_clock, wait_clock):
        from concourse.tile import ScopedClock

        nc = self.nc
        drain_inst = nc.sync.drain()
        wait_clock.add_sem_waits(
            drain_inst.ins, ScopedClock({None: tick_clock.global_clock})
        )
        assert self.sems is not None
        popped = nc._tile_sem_poison_stack.pop()
        assert popped is self._sem_poison

D = 256
H = 4
DK = 64
NV = 40000
NE = 320000
B = 64
NCORES = 8
P = 128
NPC = B // NCORES  # nodes (output rows) per core: 8
CAP = 16           # edge slots per node in the fast path

F32 = mybir.dt.float32
I32 = mybir.dt.int32
BF16 = mybir.dt.bfloat16

# hdr column layout (f32 columns; bf16/i32 fields are bitcast views)
C_IDX = 0                 # [128, 1] i32 bits: x row id per slot
C_NEGB = 1                # [128, 1] f32: exp bias (0 valid, -100 invalid)
C_EPS = 2                 # [128, 1] f32: 1e-30 (denominator guard bias)
C_QT = 3                  # [128, 2*4] : qT bf16 chunks t=0,1, each [128,8]bf16
C_EXPJ = 11               # [128, 4]   : expj bf16 [128,8] (slot->node lhsT)
C_EXPJ4 = 15              # [128, 16]  : expj replicated per head [128,32]bf16
HDR_W = 31

# wkv column layout (bf16): kv chunk0 | kv chunk1 | ident | WK blocks
C_KV = 0                  # 2 chunks of [wkt_t | wvt_t], 512 cols each
C_ID = 4 * D              # [128, 128] identity
C_WKRAW = 4 * D + P       # 4 blocks WK[t*128:(t+1)*128, c*128:(c+1)*128]
WKV_W = 4 * D + P + 4 * P

_cache: dict = {}

last_results = None  # BassKernelResults of the most recent run (for harness)


def _bf16_pack(a):
    """Pack a 2-D bf16-castable array into f32-bit columns (pairs of bf16)."""
    u = np.asarray(a, dtype=ml_dtypes.bfloat16).view(np.uint16)
    r, c = u.shape
    assert c % 2 == 0
    w = u.reshape(r, c // 2, 2)
    packed = w[:, :, 0].astype(np.uint32) | (w[:, :, 1].astype(np.uint32) << 16)
    return packed.view(np.float32)


def _build_fast(warm1=16, warm3=14):
    """Fast-path SPMD program (glob_idx == arange(64), cap 16)."""
    nc = bacc.Bacc("TRN2", target_bir_lowering=False, debug=False,
                   num_devices=NCORES)

    x_d = nc.dram_tensor("xbf", [NV, D], BF16, kind="ExternalInput")
    hdr_d = nc.dram_tensor("hdr", [P, HDR_W], F32, kind="ExternalInput")
    wq_d = nc.dram_tensor("wq", [P, 2 * D], BF16, kind="ExternalInput")
    wkv_d = nc.dram_tensor("wkv", [P, WKV_W], BF16, kind="ExternalInput")
    wo_d = nc.dram_tensor("wo", [P, 2 * D], BF16, kind="ExternalInput")
    out_d = nc.dram_tensor("out_r", [NPC, D], F32, kind="ExternalOutput")

    with _SlimTailTileContext(nc) as tc:
        with (
            tc.tile_pool(name="sbuf", bufs=1) as sb,
            tc.tile_pool(name="psum", bufs=1, space="PSUM") as pp,
        ):
            ps = pp
            # ---- input DMAs.  sync ring: hdr (gather offsets) then wq.
            # scalar ring: wkv (ident + WK blocks + K|V wall) then wo.
            hdr = sb.tile([P, HDR_W], F32, tag="hdr")
            nc.sync.dma_start(out=hdr[:], in_=hdr_d[:])
            wq = sb.tile([P, 2 * D], BF16, tag="wq")
            nc.sync.dma_start(out=wq[:], in_=wq_d[:])
            wkv = sb.tile([P, WKV_W], BF16, tag="wkv")
            nc.scalar.dma_start(out=wkv[:], in_=wkv_d[:])
            wo = sb.tile([P, 2 * D], BF16, tag="wo")
            nc.scalar.dma_start(out=wo[:], in_=wo_d[:])
            ident = wkv[:, C_ID:C_ID + P]

            # ---- the only data-dependent step: gather the slots' x rows
            xsel = sb.tile([P, D], BF16, tag="xsel")
            nc.gpsimd.indirect_dma_start(
                out=xsel[:], out_offset=None, in_=x_d[:],
                in_offset=IndirectOffsetOnAxis(ap=hdr[:, 0:1].bitcast(I32),
                                               axis=0))

            # Qtilde starts zeroed; only the in-head row blocks get written
            qtil = sb.tile([P, 2 * 32], BF16, tag="qtil")
            nc.gpsimd.memset(qtil[:], 0.0)

            # ---- PE warmup: dummy matmuls on scratch data ramp the clock
            # while the input DMAs are in flight (results are unused)
            scr_l = sb.tile([P, NPC], BF16, tag="scr_l")
            nc.gpsimd.memset(scr_l[:], 0.0)
            scr_r = sb.tile([P, 2 * D], BF16, tag="scr_r")
            nc.gpsimd.memset(scr_r[:], 0.0)
            warm_ps = ps.tile([NPC, 2 * D], F32, tag="ps_warm")
            r_ps = warm_ps[:, 0:D]
            for _ in range(warm1):
                nc.tensor.matmul(out=warm_ps[:, 0:D], lhsT=scr_l[:, 0:NPC],
                                 rhs=scr_r[:, 0:D], start=True, stop=True)

            # ---- qmT[o, n] = (query @ WQ.T).T for my 8 nodes (2 o-halves)
            qmt_ps = ps.tile([P, 2 * NPC], F32, tag="ps_qmt")
            for u in range(2):
                for t in range(2):
                    nc.tensor.matmul(
                        out=qmt_ps[:, u * NPC:(u + 1) * NPC],
                        lhsT=wq[:, t * D + u * P:t * D + (u + 1) * P],
                        rhs=hdr[:, C_QT + 4 * t:C_QT + 4 * (t + 1)]
                        .bitcast(BF16),
                        start=(t == 0), stop=(t == 1))
            # scatter head-row blocks of qmT into the (h-major) Qtilde rhs
            for u in range(2):
                for hh in range(2):
                    h = 2 * u + hh
                    eng = nc.vector if hh == 0 else nc.scalar
                    dst = qtil[hh * DK:(hh + 1) * DK,
                               u * 32 + h * NPC:u * 32 + (h + 1) * NPC]
                    src = qmt_ps[hh * DK:(hh + 1) * DK,
                                 u * NPC:(u + 1) * NPC]
                    if hh == 0:
                        eng.tensor_copy(out=dst, in_=src)
                    else:
                        eng.copy(out=dst, in_=src)

            # ---- T[i, (h,n)] = sum_o WK[o, i] * Qtilde[o, (h,n)]
            t_ps = pp.tile([P, 2 * 32], F32, tag="ps_t")
            s_ps = t_ps[:, 0:32]
            for c in range(2):
                for t in range(2):
                    nc.tensor.matmul(
                        out=t_ps[:, c * 32:(c + 1) * 32],
                        lhsT=wkv[:, C_WKRAW + (2 * t + c) * P:
                                 C_WKRAW + (2 * t + c + 1) * P],
                        rhs=qtil[:, t * 32:(t + 1) * 32],
                        start=(t == 0), stop=(t == 1))
            t_sb = sb.tile([P, 2 * 32], BF16, tag="t_sb")
            nc.scalar.copy(out=t_sb[:], in_=t_ps[:])

            # more warmup while the gather finishes
            for _ in range(warm3):
                nc.tensor.matmul(out=warm_ps[:, 0:P], lhsT=scr_l[:, 0:NPC],
                                 rhs=scr_r[:, 0:P], start=True, stop=True)

            # ---- x_sel^T (bf16), then scores + fused K-free S | V path
            xt_ps = pp.tile([P, D], BF16, tag="ps_xt")
            xt = sb.tile([P, D], BF16, tag="xt")
            for t in range(2):
                nc.tensor.transpose(out=xt_ps[:, t * P:(t + 1) * P],
                                    in_=xsel[:, t * P:(t + 1) * P],
                                    identity=ident)
                if t == 0:
                    nc.vector.tensor_copy(out=xt[:, t * P:(t + 1) * P],
                                          in_=xt_ps[:, t * P:(t + 1) * P])
                else:
                    nc.scalar.copy(out=xt[:, t * P:(t + 1) * P],
                                   in_=xt_ps[:, t * P:(t + 1) * P])
            for c in range(2):
                nc.tensor.matmul(out=s_ps,
                                 lhsT=xt[:, c * P:(c + 1) * P],
                                 rhs=t_sb[:, c * 32:(c + 1) * 32],
                                 start=(c == 0), stop=(c == 1))
            v_ps = pp.tile([P, D], F32, tag="ps_v")
            for c in range(2):
                nc.tensor.matmul(out=v_ps[:],
                                 lhsT=xt[:, c * P:(c + 1) * P],
                                 rhs=wkv[:, c * 2 * D + D:(c + 1) * 2 * D],
                                 start=(c == 0), stop=(c == 1))

            # ---- select own node's score column, then masked exp
            sel = sb.tile([P, 32], F32, tag="sel")
            nc.vector.tensor_mul(out=sel[:], in0=s_ps,
                                 in1=hdr[:, C_EXPJ4:C_EXPJ4 + 16]
                                 .bitcast(BF16))
            s = sb.tile([P, H], F32, tag="s")
            nc.vector.tensor_reduce(
                out=s[:], in_=sel[:].rearrange("p (h n) -> p h n", h=H),
                axis=mybir.AxisListType.X, op=mybir.AluOpType.add)
            agg = sb.tile([P, D + H], BF16, tag="agg")
            nc.scalar.activation(out=agg[:, D:D + H], in_=s[:],
                                 func=mybir.ActivationFunctionType.Exp,
                                 bias=hdr[:, C_NEGB:C_NEGB + 1],
                                 scale=float(1.0 / np.sqrt(DK)))
            nc.vector.tensor_tensor(
                out=agg[:, 0:D].rearrange("p (h d) -> p h d", h=H),
                in0=v_ps[:].rearrange("p (h d) -> p h d", h=H),
                in1=agg[:, D:D + H].to_broadcast([P, H, DK]),
                op=mybir.AluOpType.mult)

            # ---- per-node reduction: [numer | denom]
            acc_ps = ps.tile([NPC, D + H], F32, tag="ps_acc")
            nc.tensor.matmul(out=acc_ps[:],
                             lhsT=hdr[:, C_EXPJ:C_EXPJ + 4].bitcast(BF16),
                             rhs=agg[:], start=True, stop=True)

            # ---- normalize (+1e-30 bias so empty nodes give exact zeros)
            den = sb.tile([NPC, H], F32, tag="den")
            nc.vector.tensor_scalar_add(den[:], acc_ps[:, D:D + H], 1e-30)
            rec = sb.tile([NPC, H], F32, tag="rec")
            nc.vector.reciprocal(out=rec[:], in_=den[:])
            onode = sb.tile([NPC, D], BF16, tag="onode")
            nc.vector.tensor_tensor(
                out=onode[:].rearrange("p (h d) -> p h d", h=H),
                in0=acc_ps[:, 0:D].rearrange("p (h d) -> p h d", h=H),
                in1=rec[:].to_broadcast([NPC, H, DK]),
                op=mybir.AluOpType.mult)

            # ---- r = out_node @ WO.T
            ot_ps = ps.tile([P, 2 * NPC], BF16, tag="ps_ot")
            for t in range(2):
                nc.tensor.transpose(out=ot_ps[:, t * NPC:(t + 1) * NPC],
                                    in_=onode[:, t * P:(t + 1) * P],
                                    identity=ident[0:NPC, 0:NPC])
            ot = sb.tile([P, 2 * NPC], BF16, tag="ot")
            nc.vector.tensor_copy(out=ot[:], in_=ot_ps[:])
            for t in range(2):
                nc.tensor.matmul(out=r_ps,
                                 lhsT=ot[:, t * NPC:(t + 1) * NPC],
                                 rhs=wo[:, t * D:(t + 1) * D],
                                 start=(t == 0), stop=(t == 1))
            r_sb = sb.tile([NPC, D], F32, tag="r_sb")
            nc.vector.tensor_copy(out=r_sb[:], in_=r_ps)
            nc.sync.dma_start(out=out_d[:], in_=r_sb[:])

    nc.compile()
    return nc


def kernel(query, x, WQ, WK, WV, WO, src, dst, glob_idx):
    global last_results
    query = np.ascontiguousarray(np.asarray(query, dtype=np.float32))
    x = np.ascontiguousarray(np.asarray(x, dtype=np.float32))
    src32 = np.asarray(src, dtype=np.int32)
    dst32 = np.asarray(dst, dtype=np.int32)
    glob = np.asarray(glob_idx, dtype=np.int32)
    WQ = np.asarray(WQ, np.float32)
    WK = np.asarray(WK, np.float32)
    WV = np.asarray(WV, np.float32)
    WO = np.asarray(WO, np.float32)

    # per-global-node edge counts (for capacity + fast-path check)
    rel = dst32 < B
    gc = np.bincount(dst32[rel], minlength=B) if rel.any() else \
        np.zeros(B, np.int64)

    fast = (np.array_equal(glob, np.arange(B, dtype=glob.dtype))
            and (gc.max() <= CAP if len(gc) else True)
            and not bool(int(os.environ.get("BASSK_FORCE_GENERAL", "0"))))

    if fast:
        res = _run_fast(query, x, src32, dst32, WQ, WK, WV, WO)
    else:
        perm = np.argsort(dst32, kind="stable")
        sorted_src = np.ascontiguousarray(src32[perm])
        sorted_dst = dst32[perm]
        row_ptr = np.searchsorted(sorted_dst,
                                  np.arange(NV + 1)).astype(np.int32)
        gcnt = int((row_ptr[glob + 1] - row_ptr[glob]).max()) if len(glob) \
            else 0
        cap = 16
        while cap < gcnt:
            cap *= 2
        res = _run_general(query, x, sorted_src, row_ptr, glob, cap,
                           WQ, WK, WV, WO)
    last_results = res
    outs = [res.results[c]["out_r"] for c in range(NCORES)]
    return np.ascontiguousarray(
        np.stack(outs, axis=1).reshape(B, D).astype(np.float32))


def _run_fast(query, x, src32, dst32, WQ, WK, WV, WO):
    # only edges into the 64 global nodes matter; sort those by dst
    rel = np.flatnonzero(dst32 < B)
    r_dst = dst32[rel]
    order = np.argsort(r_dst, kind="stable")
    s_dst = r_dst[order]
    s_src = src32[rel][order]

    ident = np.eye(P, dtype=np.float32)
    wall_wq = np.empty((P, 2 * D), np.float32)
    wall_kv = np.empty((P, WKV_W), np.float32)
    wall_wo = np.empty((P, 2 * D), np.float32)
    wqt, wkt, wvt, wot = WQ.T, WK.T, WV.T, WO.T
    for t in range(2):
        dd = slice(t * P, (t + 1) * P)
        wall_wq[:, t * D:(t + 1) * D] = wqt[dd]
        wall_kv[:, t * 2 * D:t * 2 * D + D] = wkt[dd]
        wall_kv[:, t * 2 * D + D:(t + 1) * 2 * D] = wvt[dd]
        wall_wo[:, t * D:(t + 1) * D] = wot[dd]
    wall_kv[:, C_ID:C_ID + P] = ident
    for t in range(2):
        for c in range(2):
            wall_kv[:, C_WKRAW + (2 * t + c) * P:
                    C_WKRAW + (2 * t + c + 1) * P] = \
                WK[t * P:(t + 1) * P, c * P:(c + 1) * P]

    bf = ml_dtypes.bfloat16
    shared = dict(
        xbf=np.ascontiguousarray(x.astype(bf)),
        wq=np.ascontiguousarray(wall_wq.astype(bf)),
        wkv=np.ascontiguousarray(wall_kv.astype(bf)),
        wo=np.ascontiguousarray(wall_wo.astype(bf)),
    )

    # expanders: slot p belongs to node j = p // CAP
    j_of_p = np.arange(P) // CAP
    expj = np.zeros((P, NPC), np.float32)
    expj[np.arange(P), j_of_p] = 1.0
    expj4 = np.tile(expj, (1, H))  # [128, 32], h-major

    qT = query.T  # (D, B)
    in_maps = []
    for c in range(NCORES):
        my_nodes = c + NCORES * np.arange(NPC)
        lo = np.searchsorted(s_dst, my_nodes)
        hi = np.searchsorted(s_dst, my_nodes + 1)
        offs = lo[j_of_p] + np.arange(P) % CAP
        valid = offs < hi[j_of_p]
        idx = np.where(valid, s_src[np.minimum(offs, len(s_src) - 1)]
                       if len(s_src) else 0, 0).astype(np.int32)

        hdr = np.zeros((P, HDR_W), np.float32)
        hdr[:, C_IDX] = idx.view(np.float32)
        hdr[:, C_NEGB] = np.where(valid, 0.0, -100.0).astype(np.float32)
        hdr[:, C_EPS] = 1e-30
        for t in range(2):
            hdr[:, C_QT + 4 * t:C_QT + 4 * (t + 1)] = _bf16_pack(
                qT[t * P:(t + 1) * P, c::NCORES])
        hdr[:, C_EXPJ:C_EXPJ + 4] = _bf16_pack(expj)
        hdr[:, C_EXPJ4:C_EXPJ4 + 16] = _bf16_pack(expj4)
        in_maps.append(dict(shared, hdr=np.ascontiguousarray(hdr)))

    key = "fastv3"
    if key not in _cache:
        _cache[key] = _build_fast()
    nc = _cache[key]

    trace = bool(int(os.environ.get("BASSK_TRACE", "0")))
    return run_bass_kernel_spmd(nc, in_maps, core_ids=list(range(NCORES)),
                                trace=trace)


# ---------------------------------------------------------------------------
# general fallback (from validated v1 program)
# ---------------------------------------------------------------------------

def _expanders(cap):
    nslots = NPC * cap
    nch = nslots // P
    npc_chunk = P // cap
    expjt = np.zeros((NPC, P * nch), np.float32)
    expj = np.zeros((P, NPC * nch), np.float32)
    for k in range(nch):
        j_of_p = np.arange(P) // cap + k * npc_chunk
        expjt[j_of_p, k * P + np.arange(P)] = 1.0
        expj[np.arange(P), k * NPC + j_of_p] = 1.0
    woff = (np.arange(P) % cap).astype(np.float32)
    return expjt, expj, woff, nch


def _build_general(cap: int):
    """Build the SPMD Bass program. cap = edge slots per node (power of two,
    NPC*cap multiple of 128)."""
    nslots = NPC * cap
    n_chunks = nslots // P
    assert nslots % P == 0

    nc = bacc.Bacc("TRN2", target_bir_lowering=False, debug=False,
                   num_devices=NCORES)

    # ---- DRAM I/O ----
    x_d = nc.dram_tensor("x", [NV, D], F32, kind="ExternalInput")
    srcs_d = nc.dram_tensor("srcs", [NE + cap, 1], I32, kind="ExternalInput")
    rp_d = nc.dram_tensor("row_ptr", [NV + 1, 1], I32, kind="ExternalInput")
    qy_d = nc.dram_tensor("query", [B, D], F32, kind="ExternalInput")
    wqt_d = nc.dram_tensor("wqt", [D, D], F32, kind="ExternalInput")
    wkt_d = nc.dram_tensor("wkt", [D, D], F32, kind="ExternalInput")
    wvt_d = nc.dram_tensor("wvt", [D, D], F32, kind="ExternalInput")
    wot_d = nc.dram_tensor("wot", [D, D], F32, kind="ExternalInput")
    sel_d = nc.dram_tensor("sel", [B, NPC], F32, kind="ExternalInput")
    expjt_d = nc.dram_tensor("expjt", [NPC, P * n_chunks], F32,
                             kind="ExternalInput")
    expj_d = nc.dram_tensor("expj", [P, NPC * n_chunks], F32,
                            kind="ExternalInput")
    woff_d = nc.dram_tensor("win_off", [P, 1], F32, kind="ExternalInput")
    ident_d = nc.dram_tensor("ident", [P, P], F32, kind="ExternalInput")
    mgs_d = nc.dram_tensor("my_glob_s", [NPC, 1], I32, kind="ExternalInput")
    mge_d = nc.dram_tensor("my_glob_e", [NPC, 1], I32, kind="ExternalInput")
    out_d = nc.dram_tensor("out_r", [NPC, D], F32, kind="ExternalOutput")

    with _SlimTailTileContext(nc) as tc:
        with (
            tc.tile_pool(name="const", bufs=1) as cpool,
            tc.tile_pool(name="work", bufs=1) as wpool,
            tc.tile_pool(name="psum", bufs=1, space="PSUM") as ppool,
            tc.tile_pool(name="psum_small", bufs=2, space="PSUM") as spool,
        ):
            # ---- constant / weight loads (issued early, overlap the chain) --
            qy = cpool.tile([B, D], F32, tag="qy")
            nc.sync.dma_start(out=qy[:], in_=qy_d[:])
            wq = cpool.tile([P, 2 * D], F32, tag="wq")  # [d-chunk t] at cols t*D
            wk = cpool.tile([P, 2 * D], F32, tag="wk")
            wv = cpool.tile([P, 2 * D], F32, tag="wv")
            wo = cpool.tile([P, 2 * D], F32, tag="wo")
            for t in range(2):
                nc.sync.dma_start(out=wq[:, t * D:(t + 1) * D],
                                  in_=wqt_d[t * P:(t + 1) * P, :])
                nc.sync.dma_start(out=wk[:, t * D:(t + 1) * D],
                                  in_=wkt_d[t * P:(t + 1) * P, :])
                nc.sync.dma_start(out=wv[:, t * D:(t + 1) * D],
                                  in_=wvt_d[t * P:(t + 1) * P, :])
                nc.sync.dma_start(out=wo[:, t * D:(t + 1) * D],
                                  in_=wot_d[t * P:(t + 1) * P, :])
            sel = cpool.tile([B, NPC], F32, tag="sel")
            nc.sync.dma_start(out=sel[:], in_=sel_d[:])
            expjt = cpool.tile([NPC, P * n_chunks], F32, tag="expjt")
            nc.sync.dma_start(out=expjt[:], in_=expjt_d[:])
            expj = cpool.tile([P, NPC * n_chunks], F32, tag="expj")
            nc.sync.dma_start(out=expj[:], in_=expj_d[:])
            woff = cpool.tile([P, 1], F32, tag="woff")
            nc.sync.dma_start(out=woff[:], in_=woff_d[:])
            ident = cpool.tile([P, P], F32, tag="ident")
            nc.sync.dma_start(out=ident[:], in_=ident_d[:])
            mgs = cpool.tile([NPC, 1], I32, tag="mgs")
            nc.sync.dma_start(out=mgs[:], in_=mgs_d[:])
            mge = cpool.tile([NPC, 1], I32, tag="mge")
            nc.sync.dma_start(out=mge[:], in_=mge_d[:])

            # ---- row_ptr[glob] and row_ptr[glob+1] (one indirect gather) ----
            st_i = wpool.tile([NPC, 1], I32, tag="st_i")
            nc.gpsimd.indirect_dma_start(
                out=st_i[:], out_offset=None, in_=rp_d[:],
                in_offset=IndirectOffsetOnAxis(ap=mgs[:], axis=0))
            en_i = wpool.tile([NPC, 1], I32, tag="en_i")
            nc.gpsimd.indirect_dma_start(
                out=en_i[:], out_offset=None, in_=rp_d[:],
                in_offset=IndirectOffsetOnAxis(ap=mge[:], axis=0))
            st_f = wpool.tile([NPC, 1], F32, tag="st_f")
            nc.vector.tensor_copy(out=st_f[:], in_=st_i[:])
            en_f = wpool.tile([NPC, 1], F32, tag="en_f")
            nc.vector.tensor_copy(out=en_f[:], in_=en_i[:])

            # ---- q_glob = query @ WQ.T ; q_mine = my 8 rows ----
            qyt = wpool.tile([P, 2 * B], F32, tag="qyt")  # query^T d-chunks
            for t in range(2):
                pt = spool.tile([P, B], F32, tag="ps_small")
                nc.tensor.transpose(out=pt[:], in_=qy[:, t * P:(t + 1) * P],
                                    identity=ident[:B, :B])
                nc.vector.tensor_copy(out=qyt[:, t * B:(t + 1) * B], in_=pt[:])
            qg_ps = ppool.tile([B, D], F32, tag="ps_qg")
            for t in range(2):
                nc.tensor.matmul(out=qg_ps[:], lhsT=qyt[:, t * B:(t + 1) * B],
                                 rhs=wq[:, t * D:(t + 1) * D],
                                 start=(t == 0), stop=(t == 1))
            qg = wpool.tile([B, D], F32, tag="qg")
            nc.vector.tensor_copy(out=qg[:], in_=qg_ps[:])
            qm_ps = spool.tile([NPC, D], F32, tag="ps_small")
            nc.tensor.matmul(out=qm_ps[:], lhsT=sel[:], rhs=qg[:],
                             start=True, stop=True)
            qm = wpool.tile([NPC, D], F32, tag="qm")
            nc.vector.tensor_copy(out=qm[:], in_=qm_ps[:])

            # ---- accumulator over chunks (numer | denom | count) ----
            acc = wpool.tile([NPC, D + H + 1], F32, tag="acc")

            for k in range(n_chunks):
                ejt = expjt[:, k * P:(k + 1) * P]        # [NPC, P] lhsT
                ej = expj[:, k * NPC:(k + 1) * NPC]      # [P, NPC] lhsT

                # per-slot start/end expansion
                st_ps = spool.tile([P, 1], F32, tag="ps_small")
                en_ps = spool.tile([P, 1], F32, tag="ps_small")
                nc.tensor.matmul(out=st_ps[:], lhsT=ejt, rhs=st_f[:],
                                 start=True, stop=True)
                nc.tensor.matmul(out=en_ps[:], lhsT=ejt, rhs=en_f[:],
                                 start=True, stop=True)
                offs_f = wpool.tile([P, 1], F32, tag="offs_f")
                nc.vector.tensor_add(out=offs_f[:], in0=st_ps[:], in1=woff[:])
                valid = wpool.tile([P, 1], F32, tag="valid")
                nc.vector.tensor_tensor(out=valid[:], in0=offs_f[:],
                                        in1=en_ps[:], op=mybir.AluOpType.is_lt)
                offs_i = wpool.tile([P, 1], I32, tag="offs_i")
                nc.vector.tensor_copy(out=offs_i[:], in_=offs_f[:])

                # gather src ids, then x rows
                srcv = wpool.tile([P, 1], I32, tag="srcv")
                nc.gpsimd.indirect_dma_start(
                    out=srcv[:], out_offset=None, in_=srcs_d[:],
                    in_offset=IndirectOffsetOnAxis(ap=offs_i[:], axis=0))
                xsel = wpool.tile([P, D], F32, tag="xsel")
                nc.gpsimd.indirect_dma_start(
                    out=xsel[:], out_offset=None, in_=x_d[:],
                    in_offset=IndirectOffsetOnAxis(ap=srcv[:], axis=0))

                # x_sel^T (two 128x128 transposes)
                xt = wpool.tile([P, D], F32, tag="xt")
                for t in range(2):
                    xt_ps = spool.tile([P, P], F32, tag="ps_small")
                    nc.tensor.transpose(out=xt_ps[:],
                                        in_=xsel[:, t * P:(t + 1) * P],
                                        identity=ident[:])
                    nc.vector.tensor_copy(out=xt[:, t * P:(t + 1) * P],
                                          in_=xt_ps[:])

                # K/V projections of gathered rows
                k_ps = ppool.tile([P, D], F32, tag="ps_k")
                v_ps = ppool.tile([P, D], F32, tag="ps_v")
                for t in range(2):
                    nc.tensor.matmul(out=k_ps[:], lhsT=xt[:, t * P:(t + 1) * P],
                                     rhs=wk[:, t * D:(t + 1) * D],
                                     start=(t == 0), stop=(t == 1))
                for t in range(2):
                    nc.tensor.matmul(out=v_ps[:], lhsT=xt[:, t * P:(t + 1) * P],
                                     rhs=wv[:, t * D:(t + 1) * D],
                                     start=(t == 0), stop=(t == 1))
                ksel = wpool.tile([P, D], F32, tag="ksel")
                nc.vector.tensor_copy(out=ksel[:], in_=k_ps[:])
                vsel = wpool.tile([P, D], F32, tag="vsel")
                nc.vector.tensor_copy(out=vsel[:], in_=v_ps[:])

                # qe = q row per slot
                qe_ps = ppool.tile([P, D], F32, tag="ps_qe")
                nc.tensor.matmul(out=qe_ps[:], lhsT=ejt, rhs=qm[:],
                                 start=True, stop=True)

                # scores s[p,h], e = exp(s/8) * valid
                prod = wpool.tile([P, D], F32, tag="prod")
                nc.vector.tensor_mul(out=prod[:], in0=ksel[:], in1=qe_ps[:])
                s = wpool.tile([P, H], F32, tag="s")
                nc.vector.tensor_reduce(
                    out=s[:], in_=prod[:].rearrange("p (h d) -> p h d", h=H),
                    axis=mybir.AxisListType.X, op=mybir.AluOpType.add)
                e = wpool.tile([P, H], F32, tag="e")
                nc.scalar.activation(out=e[:], in_=s[:],
                                     func=mybir.ActivationFunctionType.Exp,
                                     scale=float(1.0 / np.sqrt(DK)))
                agg = wpool.tile([P, D + H + 1], F32, tag="agg")
                nc.vector.tensor_scalar_mul(agg[:, D:D + H], e[:], valid[:])
                nc.vector.tensor_copy(out=agg[:, D + H:D + H + 1], in_=valid[:])
                # w = v * alpha-weights (per head)
                for h in range(H):
                    nc.vector.tensor_scalar_mul(
                        agg[:, h * DK:(h + 1) * DK],
                        vsel[:, h * DK:(h + 1) * DK],
                        agg[:, D + h:D + h + 1])
                # per-node reduction (numer | denom | count)
                agg_ps = spool.tile([NPC, D + H + 1], F32, tag="ps_small")
                nc.tensor.matmul(out=agg_ps[:], lhsT=ej, rhs=agg[:],
                                 start=True, stop=True)
                if n_chunks == 1:
                    nc.vector.tensor_copy(out=acc[:], in_=agg_ps[:])
                elif k == 0:
                    nc.vector.tensor_copy(out=acc[:], in_=agg_ps[:])
                else:
                    nc.vector.tensor_add(out=acc[:], in0=acc[:], in1=agg_ps[:])

            # ---- normalize: out_node = numer / max(denom, empty-guard) ----
            iszero = wpool.tile([NPC, 1], F32, tag="iszero")
            nc.vector.tensor_scalar(out=iszero[:], in0=acc[:, D + H:D + H + 1],
                                    scalar1=0.5, scalar2=None,
                                    op0=mybir.AluOpType.is_lt)
            den = wpool.tile([NPC, H], F32, tag="den")
            nc.vector.tensor_scalar(out=den[:], in0=acc[:, D:D + H],
                                    scalar1=iszero[:], scalar2=None,
                                    op0=mybir.AluOpType.add)
            rec = wpool.tile([NPC, H], F32, tag="rec")
            nc.vector.reciprocal(out=rec[:], in_=den[:])
            onode = wpool.tile([NPC, D], F32, tag="onode")
            for h in range(H):
                nc.vector.tensor_scalar_mul(
                    onode[:, h * DK:(h + 1) * DK],
                    acc[:, h * DK:(h + 1) * DK], rec[:, h:h + 1])

            # ---- r = out_node @ WO.T ----
            ot = wpool.tile([P, 2 * NPC], F32, tag="ot")
            for t in range(2):
                ot_ps = spool.tile([P, NPC], F32, tag="ps_small")
                nc.tensor.transpose(out=ot_ps[:],
                                    in_=onode[:, t * P:(t + 1) * P],
                                    identity=ident[:NPC, :NPC])
                nc.vector.tensor_copy(out=ot[:, t * NPC:(t + 1) * NPC],
                                      in_=ot_ps[:])
            r_ps = spool.tile([NPC, D], F32, tag="ps_small")
            for t in range(2):
                nc.tensor.matmul(out=r_ps[:], lhsT=ot[:, t * NPC:(t + 1) * NPC],
                                 rhs=wo[:, t * D:(t + 1) * D],
                                 start=(t == 0), stop=(t == 1))
            r_sb = wpool.tile([NPC, D], F32, tag="r_sb")
            nc.vector.tensor_copy(out=r_sb[:], in_=r_ps[:])
            nc.sync.dma_start(out=out_d[:], in_=r_sb[:])

    nc.compile()
    return nc


def _run_general(query, x, sorted_src, row_ptr, glob, cap, WQ, WK, WV, WO):
    """General fallback: arbitrary glob_idx values / larger caps."""
    expjt, expj, woff, nch = _expanders(cap)
    srcs_pad = np.concatenate(
        [sorted_src, np.zeros(cap, np.int32)]).reshape(NE + cap, 1)
    rp2 = np.ascontiguousarray(row_ptr.reshape(NV + 1, 1))
    shared = dict(
        x=x, srcs=srcs_pad, row_ptr=rp2, query=query,
        wqt=np.ascontiguousarray(WQ.T), wkt=np.ascontiguousarray(WK.T),
        wvt=np.ascontiguousarray(WV.T), wot=np.ascontiguousarray(WO.T),
        expjt=expjt, expj=expj,
        win_off=np.ascontiguousarray(woff.reshape(P, 1)),
        ident=np.eye(P, dtype=np.float32))

    in_maps = []
    for c in range(NCORES):
        mine = glob[c::NCORES]
        mgs = mine.astype(np.int32).reshape(NPC, 1)
        mge = (mine + 1).astype(np.int32).reshape(NPC, 1)
        selc = np.zeros((B, NPC), np.float32)
        selc[c + NCORES * np.arange(NPC), np.arange(NPC)] = 1.0
        in_maps.append(dict(shared, my_glob_s=mgs, my_glob_e=mge, sel=selc))

    key = ("gen", cap)
    if key not in _cache:
        _cache[key] = _build_general(cap)
    nc = _cache[key]

    trace = bool(int(os.environ.get("BASSK_TRACE", "0")))
    return run_bass_kernel_spmd(nc, in_maps, core_ids=list(range(NCORES)),
                                trace=trace)


# revision 13
# speedup vs baseline: 1.5664x; 1.0448x over previous
"""Bass/Trainium2 kernel for nn_DecoderAttention (gnn message passing).

Math: q = query @ WQ.T is scattered to the 64 global nodes (glob_idx) and is
zero everywhere else, and the output only reads out[glob_idx].  Therefore only
edges whose dst is a global node contribute to the result.  Host-side we
partition the edge list by dst and shard the 64 global nodes across the 8
cores (node list c::8 -> core c).  Each core gathers the <=16 incoming edges
of each of its 8 nodes with one indirect DMA over a bf16 copy of x, computes
the per-edge scores against WK-folded queries, does the per-node masked
softmax and V aggregation (bf16 matmuls, fp32 PSUM), and applies the output
projection for its 8 rows.

Fast path (glob_idx == arange(64), per-node edge count <= 16): the host
precomputes each slot's source row id directly (slot p = node p//16, edge
p%16), so the device's only data-dependent work is the single x-row gather.
Scores use the fold  s[p,h] = x[src_p] . T[(node_p,h)]  with
T[(n,h),:] = sum_{o in head h} q[n,o] WK[o,:], so no K projection of the
gathered rows is needed.  Invalid slots get an exp bias of -100 (flushes
their softmax weight to zero) and the denominator gets +1e-30 so empty nodes
produce exact zeros.  Dummy matmuls keep the PE busy while DMAs are in
flight so the real matmuls run at full (ramped) clock.  A general fallback
using an indirect row_ptr gather handles arbitrary glob_idx / larger caps.
"""

import os

import ml_dtypes
import numpy as np

import concourse.bacc as bacc
import concourse.mybir as mybir
from concourse.bass import IndirectOffsetOnAxis
from concourse.bass_utils import run_bass_kernel_spmd
from concourse.tile import TileContext


class _SlimTailTileContext(TileContext):
    """TileContext whose kernel tail is just a drain.

    The standard tail is drain -> barrier -> sem clears -> barrier.  The NRT
    execution epilogue zeroes the entire semaphore file after every execute,
    so the kernel's own clears are redundant; only the drain (which holds the
    NEFF open until the output DMA lands) is load-bearing."""

    def _drain_and_barrier(self, tick_clock, wait_clock):
        from concourse.tile import ScopedClock

        nc = self.nc
        drain_inst = nc.sync.drain()
        wait_clock.add_sem_waits(
            drain_inst.ins, ScopedClock({None: tick_clock.global_clock})
        )
        assert self.sems is not None
        popped = nc._tile_sem_poison_stack.pop()
        assert popped is self._sem_poison

D = 256
H = 4
DK = 64
NV = 40000
NE = 320000
B = 64
NCORES = 8
P = 128
NPC = B // NCORES  # nodes (output rows) per core: 8
CAP = 16           # edge slots per node in the fast path

F32 = mybir.dt.float32
I32 = mybir.dt.int32
BF16 = mybir.dt.bfloat16

# hdr column layout (f32 columns; bf16/i32 fields are bitcast views)
C_IDX = 0                 # [128, 1] i32 bits: x row id per slot
C_NEGB = 1                # [128, 1] f32: exp bias (0 valid, -100 invalid)
C_EPS = 2                 # [128, 1] f32: 1e-30 (denominator guard bias)
C_QT = 3                  # [128, 2*4] : qT bf16 chunks t=0,1, each [128,8]bf16
C_EXPJ = 11               # [128, 4]   : expj bf16 [128,8] (slot->node lhsT)
C_EXPJ4 = 15              # [128, 16]  : expj replicated per head [128,32]bf16
HDR_W = 31

# wkv column layout (bf16): kv chunk0 | kv chunk1 | ident | WK blocks
C_KV = 0                  # 2 chunks of [wkt_t | wvt_t], 512 cols each
C_ID = 4 * D              # [128, 128] identity
C_WKRAW = 4 * D + P       # 4 blocks WK[t*128:(t+1)*128, c*128:(c+1)*128]
WKV_W = 4 * D + P + 4 * P

_cache: dict = {}

last_results = None  # BassKernelResults of the most recent run (for harness)


def _bf16_pack(a):
    """Pack a 2-D bf16-castable array into f32-bit columns (pairs of bf16)."""
    u = np.asarray(a, dtype=ml_dtypes.bfloat16).view(np.uint16)
    r, c = u.shape
    assert c % 2 == 0
    w = u.reshape(r, c // 2, 2)
    packed = w[:, :, 0].astype(np.uint32) | (w[:, :, 1].astype(np.uint32) << 16)
    return packed.view(np.float32)


def _build_fast():
    """Fast-path SPMD program (glob_idx == arange(64), cap 16)."""
    nc = bacc.Bacc("TRN2", target_bir_lowering=False, debug=False,
                   num_devices=NCORES)

    x_d = nc.dram_tensor("xbf", [NV, D], BF16, kind="ExternalInput")
    hdr_d = nc.dram_tensor("hdr", [P, HDR_W], F32, kind="ExternalInput")
    wq_d = nc.dram_tensor("wq", [P, 2 * D], BF16, kind="ExternalInput")
    wkv_d = nc.dram_tensor("wkv", [P, WKV_W], BF16, kind="ExternalInput")
    wo_d = nc.dram_tensor("wo", [P, 2 * D], BF16, kind="ExternalInput")
    out_d = nc.dram_tensor("out_r", [NPC, D], F32, kind="ExternalOutput")

    with _SlimTailTileContext(nc) as tc:
        with (
            tc.tile_pool(name="sbuf", bufs=1) as sb,
            tc.tile_pool(name="psum", bufs=1, space="PSUM") as pp,
        ):
            ps = pp
            # ---- input DMAs.  sync ring: hdr (gather offsets) then wq.
            # scalar ring: wkv (ident + WK blocks + K|V wall) then wo.
            hdr = sb.tile([P, HDR_W], F32, tag="hdr")
            nc.sync.dma_start(out=hdr[:], in_=hdr_d[:])
            wq = sb.tile([P, 2 * D], BF16, tag="wq")
            nc.sync.dma_start(out=wq[:], in_=wq_d[:])
            wkv = sb.tile([P, WKV_W], BF16, tag="wkv")
            nc.scalar.dma_start(out=wkv[:], in_=wkv_d[:])
            wo = sb.tile([P, 2 * D], BF16, tag="wo")
            nc.sync.dma_start(out=wo[:], in_=wo_d[:])
            ident = wkv[:, C_ID:C_ID + P]

            # ---- the only data-dependent step: gather the slots' x rows
            xsel = sb.tile([P, D], BF16, tag="xsel")
            nc.gpsimd.indirect_dma_start(
                out=xsel[:], out_offset=None, in_=x_d[:],
                in_offset=IndirectOffsetOnAxis(ap=hdr[:, 0:1].bitcast(I32),
                                               axis=0))

            # Qtilde starts zeroed; only the in-head row blocks get written
            qtil = sb.tile([P, 2 * 32], BF16, tag="qtil")
            nc.gpsimd.memset(qtil[:], 0.0)

            r_ps_t = ps.tile([NPC, D], F32, tag="ps_r")
            r_ps = r_ps_t[:]

            # ---- qmT[o, n] = (query @ WQ.T).T for my 8 nodes (2 o-halves)
            qmt_ps = ps.tile([P, 2 * NPC], F32, tag="ps_qmt")
            for u in range(2):
                for t in range(2):
                    nc.tensor.matmul(
                        out=qmt_ps[:, u * NPC:(u + 1) * NPC],
                        lhsT=wq[:, t * D + u * P:t * D + (u + 1) * P],
                        rhs=hdr[:, C_QT + 4 * t:C_QT + 4 * (t + 1)]
                        .bitcast(BF16),
                        start=(t == 0), stop=(t == 1))
            # scatter head-row blocks of qmT into the (h-major) Qtilde rhs
            for u in range(2):
                for hh in range(2):
                    h = 2 * u + hh
                    dst = qtil[hh * DK:(hh + 1) * DK,
                               u * 32 + h * NPC:u * 32 + (h + 1) * NPC]
                    src = qmt_ps[hh * DK:(hh + 1) * DK,
                                 u * NPC:(u + 1) * NPC]
                    nc.vector.tensor_copy(out=dst, in_=src)

            # ---- T[i, (h,n)] = sum_o WK[o, i] * Qtilde[o, (h,n)]
            t_ps = pp.tile([P, 2 * 32], F32, tag="ps_t")
            s_ps = t_ps[:, 0:32]
            for c in range(2):
                for t in range(2):
                    nc.tensor.matmul(
                        out=t_ps[:, c * 32:(c + 1) * 32],
                        lhsT=wkv[:, C_WKRAW + (2 * t + c) * P:
                                 C_WKRAW + (2 * t + c + 1) * P],
                        rhs=qtil[:, t * 32:(t + 1) * 32],
                        start=(t == 0), stop=(t == 1))
            t_sb = sb.tile([P, 2 * 32], BF16, tag="t_sb")
            nc.vector.tensor_copy(out=t_sb[:], in_=t_ps[:])

            # ---- x_sel^T (bf16), then scores + fused K-free S | V path
            xt_ps = pp.tile([P, D], BF16, tag="ps_xt")
            xt = sb.tile([P, D], BF16, tag="xt")
            for t in range(2):
                nc.tensor.transpose(out=xt_ps[:, t * P:(t + 1) * P],
                                    in_=xsel[:, t * P:(t + 1) * P],
                                    identity=ident)
                if t == 0:
                    nc.vector.tensor_copy(out=xt[:, t * P:(t + 1) * P],
                                          in_=xt_ps[:, t * P:(t + 1) * P])
                else:
                    nc.scalar.copy(out=xt[:, t * P:(t + 1) * P],
                                   in_=xt_ps[:, t * P:(t + 1) * P])
            for c in range(2):
                nc.tensor.matmul(out=s_ps,
                                 lhsT=xt[:, c * P:(c + 1) * P],
                                 rhs=t_sb[:, c * 32:(c + 1) * 32],
                                 start=(c == 0), stop=(c == 1))
            v_ps = pp.tile([P, D], F32, tag="ps_v")
            for c in range(2):
                nc.tensor.matmul(out=v_ps[:],
                                 lhsT=xt[:, c * P:(c + 1) * P],
                                 rhs=wkv[:, c * 2 * D + D:(c + 1) * 2 * D],
                                 start=(c == 0), stop=(c == 1))

            # ---- select own node's score column, then masked exp
            sel = sb.tile([P, 32], F32, tag="sel")
            nc.vector.tensor_mul(out=sel[:], in0=s_ps,
                                 in1=hdr[:, C_EXPJ4:C_EXPJ4 + 16]
                                 .bitcast(BF16))
            s = sb.tile([P, H], F32, tag="s")
            nc.vector.tensor_reduce(
                out=s[:], in_=sel[:].rearrange("p (h n) -> p h n", h=H),
                axis=mybir.AxisListType.X, op=mybir.AluOpType.add)
            agg = sb.tile([P, D + H], BF16, tag="agg")
            nc.scalar.activation(out=agg[:, D:D + H], in_=s[:],
                                 func=mybir.ActivationFunctionType.Exp,
                                 bias=hdr[:, C_NEGB:C_NEGB + 1],
                                 scale=float(1.0 / np.sqrt(DK)))
            nc.vector.tensor_tensor(
                out=agg[:, 0:D].rearrange("p (h d) -> p h d", h=H),
                in0=v_ps[:].rearrange("p (h d) -> p h d", h=H),
                in1=agg[:, D:D + H].to_broadcast([P, H, DK]),
                op=mybir.AluOpType.mult)

            # ---- per-node reduction: [numer | denom]
            acc_ps = ps.tile([NPC, D + H], F32, tag="ps_acc")
            nc.tensor.matmul(out=acc_ps[:],
                             lhsT=hdr[:, C_EXPJ:C_EXPJ + 4].bitcast(BF16),
                             rhs=agg[:], start=True, stop=True)

            # ---- normalize (+1e-30 bias so empty nodes give exact zeros)
            den = sb.tile([NPC, H], F32, tag="den")
            nc.vector.tensor_scalar_add(den[:], acc_ps[:, D:D + H], 1e-30)
            rec = sb.tile([NPC, H], F32, tag="rec")
            nc.vector.reciprocal(out=rec[:], in_=den[:])
            onode = sb.tile([NPC, D], BF16, tag="onode")
            nc.vector.tensor_tensor(
                out=onode[:].rearrange("p (h d) -> p h d", h=H),
                in0=acc_ps[:, 0:D].rearrange("p (h d) -> p h d", h=H),
                in1=rec[:].to_broadcast([NPC, H, DK]),
                op=mybir.AluOpType.mult)

            # ---- r = out_node @ WO.T
            ot_ps = ps.tile([P, 2 * NPC], BF16, tag="ps_ot")
            for t in range(2):
                nc.tensor.transpose(out=ot_ps[:, t * NPC:(t + 1) * NPC],
                                    in_=onode[:, t * P:(t + 1) * P],
                                    identity=ident[0:NPC, 0:NPC])
            ot = sb.tile([P, 2 * NPC], BF16, tag="ot")
            nc.vector.tensor_copy(out=ot[:], in_=ot_ps[:])
            for t in range(2):
                nc.tensor.matmul(out=r_ps,
                                 lhsT=ot[:, t * NPC:(t + 1) * NPC],
                                 rhs=wo[:, t * D:(t + 1) * D],
                                 start=(t == 0), stop=(t == 1))
            r_sb = sb.tile([NPC, D], F32, tag="r_sb")
            nc.vector.tensor_copy(out=r_sb[:], in_=r_ps)
            nc.sync.dma_start(out=out_d[:], in_=r_sb[:])

    nc.compile()
    return nc


def kernel(query, x, WQ, WK, WV, WO, src, dst, glob_idx):
    global last_results
    query = np.ascontiguousarray(np.asarray(query, dtype=np.float32))
    x = np.ascontiguousarray(np.asarray(x, dtype=np.float32))
    src32 = np.asarray(src, dtype=np.int32)
    dst32 = np.asarray(dst, dtype=np.int32)
    glob = np.asarray(glob_idx, dtype=np.int32)
    WQ = np.asarray(WQ, np.float32)
    WK = np.asarray(WK, np.float32)
    WV = np.asarray(WV, np.float32)
    WO = np.asarray(WO, np.float32)

    # per-global-node edge counts (for capacity + fast-path check)
    rel = dst32 < B
    gc = np.bincount(dst32[rel], minlength=B) if rel.any() else \
        np.zeros(B, np.int64)

    fast = (np.array_equal(glob, np.arange(B, dtype=glob.dtype))
            and (gc.max() <= CAP if len(gc) else True)
            and not bool(int(os.environ.get("BASSK_FORCE_GENERAL", "0"))))

    if fast:
        res = _run_fast(query, x, src32, dst32, WQ, WK, WV, WO)
    else:
        perm = np.argsort(dst32, kind="stable")
        sorted_src = np.ascontiguousarray(src32[perm])
        sorted_dst = dst32[perm]
        row_ptr = np.searchsorted(sorted_dst,
                                  np.arange(NV + 1)).astype(np.int32)
        gcnt = int((row_ptr[glob + 1] - row_ptr[glob]).max()) if len(glob) \
            else 0
        cap = 16
        while cap < gcnt:
            cap *= 2
        res = _run_general(query, x, sorted_src, row_ptr, glob, cap,
                           WQ, WK, WV, WO)
    last_results = res
    outs = [res.results[c]["out_r"] for c in range(NCORES)]
    return np.ascontiguousarray(
        np.stack(outs, axis=1).reshape(B, D).astype(np.float32))


def _run_fast(query, x, src32, dst32, WQ, WK, WV, WO):
    # only edges into the 64 global nodes matter; sort those by dst
    rel = np.flatnonzero(dst32 < B)
    r_dst = dst32[rel]
    order = np.argsort(r_dst, kind="stable")
    s_dst = r_dst[order]
    s_src = src32[rel][order]

    ident = np.eye(P, dtype=np.float32)
    wall_wq = np.empty((P, 2 * D), np.float32)
    wall_kv = np.empty((P, WKV_W), np.float32)
    wall_wo = np.empty((P, 2 * D), np.float32)
    wqt, wkt, wvt, wot = WQ.T, WK.T, WV.T, WO.T
    for t in range(2):
        dd = slice(t * P, (t + 1) * P)
        wall_wq[:, t * D:(t + 1) * D] = wqt[dd]
        wall_kv[:, t * 2 * D:t * 2 * D + D] = wkt[dd]
        wall_kv[:, t * 2 * D + D:(t + 1) * 2 * D] = wvt[dd]
        wall_wo[:, t * D:(t + 1) * D] = wot[dd]
    wall_kv[:, C_ID:C_ID + P] = ident
    for t in range(2):
        for c in range(2):
            wall_kv[:, C_WKRAW + (2 * t + c) * P:
                    C_WKRAW + (2 * t + c + 1) * P] = \
                WK[t * P:(t + 1) * P, c * P:(c + 1) * P]

    bf = ml_dtypes.bfloat16
    shared = dict(
        xbf=np.ascontiguousarray(x.astype(bf)),
        wq=np.ascontiguousarray(wall_wq.astype(bf)),
        wkv=np.ascontiguousarray(wall_kv.astype(bf)),
        wo=np.ascontiguousarray(wall_wo.astype(bf)),
    )

    # expanders: slot p belongs to node j = p // CAP
    j_of_p = np.arange(P) // CAP
    expj = np.zeros((P, NPC), np.float32)
    expj[np.arange(P), j_of_p] = 1.0
    expj4 = np.tile(expj, (1, H))  # [128, 32], h-major

    qT = query.T  # (D, B)
    in_maps = []
    for c in range(NCORES):
        my_nodes = c + NCORES * np.arange(NPC)
        lo = np.searchsorted(s_dst, my_nodes)
        hi = np.searchsorted(s_dst, my_nodes + 1)
        offs = lo[j_of_p] + np.arange(P) % CAP
        valid = offs < hi[j_of_p]
        idx = np.where(valid, s_src[np.minimum(offs, len(s_src) - 1)]
                       if len(s_src) else 0, 0).astype(np.int32)

        hdr = np.zeros((P, HDR_W), np.float32)
        hdr[:, C_IDX] = idx.view(np.float32)
        hdr[:, C_NEGB] = np.where(valid, 0.0, -100.0).astype(np.float32)
        hdr[:, C_EPS] = 1e-30
        for t in range(2):
            hdr[:, C_QT + 4 * t:C_QT + 4 * (t + 1)] = _bf16_pack(
                qT[t * P:(t + 1) * P, c::NCORES])
        hdr[:, C_EXPJ:C_EXPJ + 4] = _bf16_pack(expj)
        hdr[:, C_EXPJ4:C_EXPJ4 + 16] = _bf16_pack(expj4)
        in_maps.append(dict(shared, hdr=np.ascontiguousarray(hdr)))

    key = "fastv3"
    if key not in _cache:
        _cache[key] = _build_fast()
    nc = _cache[key]

    trace = bool(int(os.environ.get("BASSK_TRACE", "0")))
    return run_bass_kernel_spmd(nc, in_maps, core_ids=list(range(NCORES)),
                                trace=trace)


# ---------------------------------------------------------------------------
# general fallback (from validated v1 program)
# ---------------------------------------------------------------------------

def _expanders(cap):
    nslots = NPC * cap
    nch = nslots // P
    npc_chunk = P // cap
    expjt = np.zeros((NPC, P * nch), np.float32)
    expj = np.zeros((P, NPC * nch), np.float32)
    for k in range(nch):
        j_of_p = np.arange(P) // cap + k * npc_chunk
        expjt[j_of_p, k * P + np.arange(P)] = 1.0
        expj[np.arange(P), k * NPC + j_of_p] = 1.0
    woff = (np.arange(P) % cap).astype(np.float32)
    return expjt, expj, woff, nch


def _build_general(cap: int):
    """Build the SPMD Bass program. cap = edge slots per node (power of two,
    NPC*cap multiple of 128)."""
    nslots = NPC * cap
    n_chunks = nslots // P
    assert nslots % P == 0

    nc = bacc.Bacc("TRN2", target_bir_lowering=False, debug=False,
                   num_devices=NCORES)

    # ---- DRAM I/O ----
    x_d = nc.dram_tensor("x", [NV, D], F32, kind="ExternalInput")
    srcs_d = nc.dram_tensor("srcs", [NE + cap, 1], I32, kind="ExternalInput")
    rp_d = nc.dram_tensor("row_ptr", [NV + 1, 1], I32, kind="ExternalInput")
    qy_d = nc.dram_tensor("query", [B, D], F32, kind="ExternalInput")
    wqt_d = nc.dram_tensor("wqt", [D, D], F32, kind="ExternalInput")
    wkt_d = nc.dram_tensor("wkt", [D, D], F32, kind="ExternalInput")
    wvt_d = nc.dram_tensor("wvt", [D, D], F32, kind="ExternalInput")
    wot_d = nc.dram_tensor("wot", [D, D], F32, kind="ExternalInput")
    sel_d = nc.dram_tensor("sel", [B, NPC], F32, kind="ExternalInput")
    expjt_d = nc.dram_tensor("expjt", [NPC, P * n_chunks], F32,
                             kind="ExternalInput")
    expj_d = nc.dram_tensor("expj", [P, NPC * n_chunks], F32,
                            kind="ExternalInput")
    woff_d = nc.dram_tensor("win_off", [P, 1], F32, kind="ExternalInput")
    ident_d = nc.dram_tensor("ident", [P, P], F32, kind="ExternalInput")
    mgs_d = nc.dram_tensor("my_glob_s", [NPC, 1], I32, kind="ExternalInput")
    mge_d = nc.dram_tensor("my_glob_e", [NPC, 1], I32, kind="ExternalInput")
    out_d = nc.dram_tensor("out_r", [NPC, D], F32, kind="ExternalOutput")

    with _SlimTailTileContext(nc) as tc:
        with (
            tc.tile_pool(name="const", bufs=1) as cpool,
            tc.tile_pool(name="work", bufs=1) as wpool,
            tc.tile_pool(name="psum", bufs=1, space="PSUM") as ppool,
            tc.tile_pool(name="psum_small", bufs=2, space="PSUM") as spool,
        ):
            # ---- constant / weight loads (issued early, overlap the chain) --
            qy = cpool.tile([B, D], F32, tag="qy")
            nc.sync.dma_start(out=qy[:], in_=qy_d[:])
            wq = cpool.tile([P, 2 * D], F32, tag="wq")  # [d-chunk t] at cols t*D
            wk = cpool.tile([P, 2 * D], F32, tag="wk")
            wv = cpool.tile([P, 2 * D], F32, tag="wv")
            wo = cpool.tile([P, 2 * D], F32, tag="wo")
            for t in range(2):
                nc.sync.dma_start(out=wq[:, t * D:(t + 1) * D],
                                  in_=wqt_d[t * P:(t + 1) * P, :])
                nc.sync.dma_start(out=wk[:, t * D:(t + 1) * D],
                                  in_=wkt_d[t * P:(t + 1) * P, :])
                nc.sync.dma_start(out=wv[:, t * D:(t + 1) * D],
                                  in_=wvt_d[t * P:(t + 1) * P, :])
                nc.sync.dma_start(out=wo[:, t * D:(t + 1) * D],
                                  in_=wot_d[t * P:(t + 1) * P, :])
            sel = cpool.tile([B, NPC], F32, tag="sel")
            nc.sync.dma_start(out=sel[:], in_=sel_d[:])
            expjt = cpool.tile([NPC, P * n_chunks], F32, tag="expjt")
            nc.sync.dma_start(out=expjt[:], in_=expjt_d[:])
            expj = cpool.tile([P, NPC * n_chunks], F32, tag="expj")
            nc.sync.dma_start(out=expj[:], in_=expj_d[:])
            woff = cpool.tile([P, 1], F32, tag="woff")
            nc.sync.dma_start(out=woff[:], in_=woff_d[:])
            ident = cpool.tile([P, P], F32, tag="ident")
            nc.sync.dma_start(out=ident[:], in_=ident_d[:])
            mgs = cpool.tile([NPC, 1], I32, tag="mgs")
            nc.sync.dma_start(out=mgs[:], in_=mgs_d[:])
            mge = cpool.tile([NPC, 1], I32, tag="mge")
            nc.sync.dma_start(out=mge[:], in_=mge_d[:])

            # ---- row_ptr[glob] and row_ptr[glob+1] (one indirect gather) ----
            st_i = wpool.tile([NPC, 1], I32, tag="st_i")
            nc.gpsimd.indirect_dma_start(
                out=st_i[:], out_offset=None, in_=rp_d[:],
                in_offset=IndirectOffsetOnAxis(ap=mgs[:], axis=0))
            en_i = wpool.tile([NPC, 1], I32, tag="en_i")
            nc.gpsimd.indirect_dma_start(
                out=en_i[:], out_offset=None, in_=rp_d[:],
                in_offset=IndirectOffsetOnAxis(ap=mge[:], axis=0))
            st_f = wpool.tile([NPC, 1], F32, tag="st_f")
            nc.vector.tensor_copy(out=st_f[:], in_=st_i[:])
            en_f = wpool.tile([NPC, 1], F32, tag="en_f")
            nc.vector.tensor_copy(out=en_f[:], in_=en_i[:])

            # ---- q_glob = query @ WQ.T ; q_mine = my 8 rows ----
            qyt = wpool.tile([P, 2 * B], F32, tag="qyt")  # query^T d-chunks
            for t in range(2):
                pt = spool.tile([P, B], F32, tag="ps_small")
                nc.tensor.transpose(out=pt[:], in_=qy[:, t * P:(t + 1) * P],
                                    identity=ident[:B, :B])
                nc.vector.tensor_copy(out=qyt[:, t * B:(t + 1) * B], in_=pt[:])
            qg_ps = ppool.tile([B, D], F32, tag="ps_qg")
            for t in range(2):
                nc.tensor.matmul(out=qg_ps[:], lhsT=qyt[:, t * B:(t + 1) * B],
                                 rhs=wq[:, t * D:(t + 1) * D],
                                 start=(t == 0), stop=(t == 1))
            qg = wpool.tile([B, D], F32, tag="qg")
            nc.vector.tensor_copy(out=qg[:], in_=qg_ps[:])
            qm_ps = spool.tile([NPC, D], F32, tag="ps_small")
            nc.tensor.matmul(out=qm_ps[:], lhsT=sel[:], rhs=qg[:],
                             start=True, stop=True)
            qm = wpool.tile([NPC, D], F32, tag="qm")
            nc.vector.tensor_copy(out=qm[:], in_=qm_ps[:])

            # ---- accumulator over chunks (numer | denom | count) ----
            acc = wpool.tile([NPC, D + H + 1], F32, tag="acc")

            for k in range(n_chunks):
                ejt = expjt[:, k * P:(k + 1) * P]        # [NPC, P] lhsT
                ej = expj[:, k * NPC:(k + 1) * NPC]      # [P, NPC] lhsT

                # per-slot start/end expansion
                st_ps = spool.tile([P, 1], F32, tag="ps_small")
                en_ps = spool.tile([P, 1], F32, tag="ps_small")
                nc.tensor.matmul(out=st_ps[:], lhsT=ejt, rhs=st_f[:],
                                 start=True, stop=True)
                nc.tensor.matmul(out=en_ps[:], lhsT=ejt, rhs=en_f[:],
                                 start=True, stop=True)
                offs_f = wpool.tile([P, 1], F32, tag="offs_f")
                nc.vector.tensor_add(out=offs_f[:], in0=st_ps[:], in1=woff[:])
                valid = wpool.tile([P, 1], F32, tag="valid")
                nc.vector.tensor_tensor(out=valid[:], in0=offs_f[:],
                                        in1=en_ps[:], op=mybir.AluOpType.is_lt)
                offs_i = wpool.tile([P, 1], I32, tag="offs_i")
                nc.vector.tensor_copy(out=offs_i[:], in_=offs_f[:])

                # gather src ids, then x rows
                srcv = wpool.tile([P, 1], I32, tag="srcv")
                nc.gpsimd.indirect_dma_start(
                    out=srcv[:], out_offset=None, in_=srcs_d[:],
                    in_offset=IndirectOffsetOnAxis(ap=offs_i[:], axis=0))
                xsel = wpool.tile([P, D], F32, tag="xsel")
                nc.gpsimd.indirect_dma_start(
                    out=xsel[:], out_offset=None, in_=x_d[:],
                    in_offset=IndirectOffsetOnAxis(ap=srcv[:], axis=0))

                # x_sel^T (two 128x128 transposes)
                xt = wpool.tile([P, D], F32, tag="xt")
                for t in range(2):
                    xt_ps = spool.tile([P, P], F32, tag="ps_small")
                    nc.tensor.transpose(out=xt_ps[:],
                                        in_=xsel[:, t * P:(t + 1) * P],
                                        identity=ident[:])
                    nc.vector.tensor_copy(out=xt[:, t * P:(t + 1) * P],
                                          in_=xt_ps[:])

                # K/V projections of gathered rows
                k_ps = ppool.tile([P, D], F32, tag="ps_k")
                v_ps = ppool.tile([P, D], F32, tag="ps_v")
                for t in range(2):
                    nc.tensor.matmul(out=k_ps[:], lhsT=xt[:, t * P:(t + 1) * P],
                                     rhs=wk[:, t * D:(t + 1) * D],
                                     start=(t == 0), stop=(t == 1))
                for t in range(2):
                    nc.tensor.matmul(out=v_ps[:], lhsT=xt[:, t * P:(t + 1) * P],
                                     rhs=wv[:, t * D:(t + 1) * D],
                                     start=(t == 0), stop=(t == 1))
                ksel = wpool.tile([P, D], F32, tag="ksel")
                nc.vector.tensor_copy(out=ksel[:], in_=k_ps[:])
                vsel = wpool.tile([P, D], F32, tag="vsel")
                nc.vector.tensor_copy(out=vsel[:], in_=v_ps[:])

                # qe = q row per slot
                qe_ps = ppool.tile([P, D], F32, tag="ps_qe")
                nc.tensor.matmul(out=qe_ps[:], lhsT=ejt, rhs=qm[:],
                                 start=True, stop=True)

                # scores s[p,h], e = exp(s/8) * valid
                prod = wpool.tile([P, D], F32, tag="prod")
                nc.vector.tensor_mul(out=prod[:], in0=ksel[:], in1=qe_ps[:])
                s = wpool.tile([P, H], F32, tag="s")
                nc.vector.tensor_reduce(
                    out=s[:], in_=prod[:].rearrange("p (h d) -> p h d", h=H),
                    axis=mybir.AxisListType.X, op=mybir.AluOpType.add)
                e = wpool.tile([P, H], F32, tag="e")
                nc.scalar.activation(out=e[:], in_=s[:],
                                     func=mybir.ActivationFunctionType.Exp,
                                     scale=float(1.0 / np.sqrt(DK)))
                agg = wpool.tile([P, D + H + 1], F32, tag="agg")
                nc.vector.tensor_scalar_mul(agg[:, D:D + H], e[:], valid[:])
                nc.vector.tensor_copy(out=agg[:, D + H:D + H + 1], in_=valid[:])
                # w = v * alpha-weights (per head)
                for h in range(H):
                    nc.vector.tensor_scalar_mul(
                        agg[:, h * DK:(h + 1) * DK],
                        vsel[:, h * DK:(h + 1) * DK],
                        agg[:, D + h:D + h + 1])
                # per-node reduction (numer | denom | count)
                agg_ps = spool.tile([NPC, D + H + 1], F32, tag="ps_small")
                nc.tensor.matmul(out=agg_ps[:], lhsT=ej, rhs=agg[:],
                                 start=True, stop=True)
                if n_chunks == 1:
                    nc.vector.tensor_copy(out=acc[:], in_=agg_ps[:])
                elif k == 0:
                    nc.vector.tensor_copy(out=acc[:], in_=agg_ps[:])
                else:
                    nc.vector.tensor_add(out=acc[:], in0=acc[:], in1=agg_ps[:])

            # ---- normalize: out_node = numer / max(denom, empty-guard) ----
            iszero = wpool.tile([NPC, 1], F32, tag="iszero")
            nc.vector.tensor_scalar(out=iszero[:], in0=acc[:, D + H:D + H + 1],
                                    scalar1=0.5, scalar2=None,
                                    op0=mybir.AluOpType.is_lt)
            den = wpool.tile([NPC, H], F32, tag="den")
            nc.vector.tensor_scalar(out=den[:], in0=acc[:, D:D + H],
                                    scalar1=iszero[:], scalar2=None,
                                    op0=mybir.AluOpType.add)
            rec = wpool.tile([NPC, H], F32, tag="rec")
            nc.vector.reciprocal(out=rec[:], in_=den[:])
            onode = wpool.tile([NPC, D], F32, tag="onode")
            for h in range(H):
                nc.vector.tensor_scalar_mul(
                    onode[:, h * DK:(h + 1) * DK],
                    acc[:, h * DK:(h + 1) * DK], rec[:, h:h + 1])

            # ---- r = out_node @ WO.T ----
            ot = wpool.tile([P, 2 * NPC], F32, tag="ot")
            for t in range(2):
                ot_ps = spool.tile([P, NPC], F32, tag="ps_small")
                nc.tensor.transpose(out=ot_ps[:],
                                    in_=onode[:, t * P:(t + 1) * P],
                                    identity=ident[:NPC, :NPC])
                nc.vector.tensor_copy(out=ot[:, t * NPC:(t + 1) * NPC],
                                      in_=ot_ps[:])
            r_ps = spool.tile([NPC, D], F32, tag="ps_small")
            for t in range(2):
                nc.tensor.matmul(out=r_ps[:], lhsT=ot[:, t * NPC:(t + 1) * NPC],
                                 rhs=wo[:, t * D:(t + 1) * D],
                                 start=(t == 0), stop=(t == 1))
            r_sb = wpool.tile([NPC, D], F32, tag="r_sb")
            nc.vector.tensor_copy(out=r_sb[:], in_=r_ps[:])
            nc.sync.dma_start(out=out_d[:], in_=r_sb[:])

    nc.compile()
    return nc


def _run_general(query, x, sorted_src, row_ptr, glob, cap, WQ, WK, WV, WO):
    """General fallback: arbitrary glob_idx values / larger caps."""
    expjt, expj, woff, nch = _expanders(cap)
    srcs_pad = np.concatenate(
        [sorted_src, np.zeros(cap, np.int32)]).reshape(NE + cap, 1)
    rp2 = np.ascontiguousarray(row_ptr.reshape(NV + 1, 1))
    shared = dict(
        x=x, srcs=srcs_pad, row_ptr=rp2, query=query,
        wqt=np.ascontiguousarray(WQ.T), wkt=np.ascontiguousarray(WK.T),
        wvt=np.ascontiguousarray(WV.T), wot=np.ascontiguousarray(WO.T),
        expjt=expjt, expj=expj,
        win_off=np.ascontiguousarray(woff.reshape(P, 1)),
        ident=np.eye(P, dtype=np.float32))

    in_maps = []
    for c in range(NCORES):
        mine = glob[c::NCORES]
        mgs = mine.astype(np.int32).reshape(NPC, 1)
        mge = (mine + 1).astype(np.int32).reshape(NPC, 1)
        selc = np.zeros((B, NPC), np.float32)
        selc[c + NCORES * np.arange(NPC), np.arange(NPC)] = 1.0
        in_maps.append(dict(shared, my_glob_s=mgs, my_glob_e=mge, sel=selc))

    key = ("gen", cap)
    if key not in _cache:
        _cache[key] = _build_general(cap)
    nc = _cache[key]

    trace = bool(int(os.environ.get("BASSK_TRACE", "0")))
    return run_bass_kernel_spmd(nc, in_maps, core_ids=list(range(NCORES)),
                                trace=trace)
